# revision 31
# baseline (speedup 1.0000x reference)
"""DVAE encoder (batched DAG GRU message passing) on 8 trn2 NeuronCores.

v3 strategy: pure data-parallel over batch (256 graphs/core). Feature-major
compute (features on partitions, batch on free dim); GRU/gate/mapper are
weight-stationary matmuls with N=256. The r,z gate GEMM runs in fp8-e4m3
DoubleRow (sigmoid paths absorb the quantization; verified well under the
accuracy budget).

Predecessor aggregation runs on the PE as diag-mask matmuls accumulating in
PSUM, with all diagonal masks prebuilt on the host and DMAed in (no
on-device mask builds). The per-vertex mask chain is split into two batches
scheduled into the PE's two natural idle windows: the mid-step GRU-pointwise
bubble and the step-boundary wait for the next hidden state, keeping the PE
continuously busy (HAM stays unthrottled). Sigmoids/tanh and PSUM->SBUF
copies run on the Scalar engine; the n/h'/message pointwise runs on DVE.
"""

import numpy as np

B, MAX_N, NVT, HS, NZ = 2048, 16, 8, 501, 56
HP = 512
NC_CORES = 8
BL = B // NC_CORES   # 256 per core
NBT = BL // 128      # 2 batch tiles

FP8_WA = False       # r,z GEMM in fp8 DoubleRow (off: bf16, same PE cost)

_CACHE = {}

# host-side mask index: (w, u, bt) -> column block, w-major so the DMA
# arrives in first-use order
def _mask_index():
    idx = {}
    k = 0
    for w in range(1, MAX_N):
        for u in range(w):
            for bt in range(NBT):
                idx[(w, u, bt)] = k
                k += 1
    return idx, k

MASK_IDX, N_MASKS = _mask_index()


def _build_nc():
    import concourse.mybir as mybir
    import concourse.tile as tile
    from concourse import bacc

    F32 = mybir.dt.float32
    BF = mybir.dt.bfloat16
    F8 = mybir.dt.float8e4
    DR = mybir.MatmulPerfMode.DoubleRow
    Sig = mybir.ActivationFunctionType.Sigmoid
    Tanh = mybir.ActivationFunctionType.Tanh
    Ident = mybir.ActivationFunctionType.Identity
    ADD = mybir.AluOpType.add
    MUL = mybir.AluOpType.mult

    nc = bacc.Bacc("TRN2", target_bir_lowering=False, debug=False,
                   num_devices=NC_CORES)

    # ---- DRAM parameters (host-prepped) ----
    d_wa = nc.dram_tensor("wa", [128, 4 * 1024], F8 if FP8_WA else BF,
                          kind="ExternalInput").ap()
    d_wb = nc.dram_tensor("wb", [128, 512], BF, kind="ExternalInput").ap()
    d_wc = nc.dram_tensor("wc", [128, 4 * 512], BF, kind="ExternalInput").ap()
    d_wg = nc.dram_tensor("wg", [128, 4 * 512], BF, kind="ExternalInput").ap()
    d_wm = nc.dram_tensor("wm", [128, 4 * 512], BF, kind="ExternalInput").ap()
    d_wf = nc.dram_tensor("wf", [128, 4 * 112], BF, kind="ExternalInput").ap()
    d_gb = nc.dram_tensor("gb", [128, 64], F32, kind="ExternalInput").ap()
    d_mb = nc.dram_tensor("mb", [128, 64], F32, kind="ExternalInput").ap()
    d_fcb = nc.dram_tensor("fcb", [128, 1], F32, kind="ExternalInput").ap()
    d_xh = nc.dram_tensor("xh", [128, MAX_N * NBT * 9], BF,
                          kind="ExternalInput").ap()
    d_dm = nc.dram_tensor("dmasks", [128, N_MASKS * 128], BF,
                          kind="ExternalInput").ap()
    d_id = nc.dram_tensor("ident", [128, 128], BF, kind="ExternalInput").ap()
    d_y = nc.dram_tensor("y", [112, BL], F32, kind="ExternalOutput").ap()

    with tile.TileContext(nc) as tc:
        with tc.tile_pool(name="wts", bufs=1) as wts, \
             tc.tile_pool(name="state", bufs=1) as state, \
             tc.tile_pool(name="hbuf", bufs=2) as hbuf, \
             tc.tile_pool(name="work", bufs=2) as work, \
             tc.tile_pool(name="gps", bufs=2, space="PSUM") as gps, \
             tc.tile_pool(name="pgg", bufs=1, space="PSUM") as paggp, \
             tc.tile_pool(name="ptr", bufs=2, space="PSUM") as ptrp:

            # ---- load static data ----
            wa = wts.tile([128, 4, 1024], F8 if FP8_WA else BF, tag="wa",
                          name="wa")
            wb = wts.tile([128, 512], BF, tag="wb", name="wb")
            wc = wts.tile([128, 4, 512], BF, tag="wc", name="wc")
            wg = wts.tile([128, 4, 512], BF, tag="wg", name="wg")
            wm = wts.tile([128, 4, 512], BF, tag="wm", name="wm")
            wf = wts.tile([128, 4, 112], BF, tag="wf", name="wf")
            gb = wts.tile([128, 64], F32, tag="gb", name="gb")
            mb = wts.tile([128, 64], F32, tag="mb", name="mb")
            fcb = wts.tile([128, 1], F32, tag="fcb", name="fcb")
            xh = wts.tile([128, MAX_N * NBT * 9], BF, tag="xh", name="xh")
            dmasks = wts.tile([128, N_MASKS * 128], BF, tag="dmasks",
                              name="dmasks")
            ident = wts.tile([128, 128], BF, tag="ident", name="ident")
            # step-0-critical tensors first
            for t, d in ((xh, d_xh), (ident, d_id), (wa, d_wa), (wb, d_wb),
                         (wc, d_wc), (gb, d_gb), (mb, d_mb), (wg, d_wg),
                         (wm, d_wm), (wf, d_wf), (fcb, d_fcb)):
                nc.sync.dma_start(out=t[:], in_=d[:])
            # masks arrive in per-vertex chunks so early steps don't wait
            # on the whole 7.9MB transfer
            for w in range(1, MAX_N):
                k0 = MASK_IDX[(w, 0, 0)]
                k1 = MASK_IDX[(w, w - 1, NBT - 1)] + 1
                nc.sync.dma_start(out=dmasks[:, k0 * 128:k1 * 128],
                                  in_=d_dm[:, k0 * 128:k1 * 128])

            # messages batch-major: [128, v, bt, 512]
            msb = state.tile([128, MAX_N, NBT, 512], BF, tag="msb",
                             name="msb")

            def mask_ap(w, u, bt):
                k = MASK_IDX[(w, u, bt)]
                return dmasks[:, k * 128:(k + 1) * 128]

            def make_hT(v, hin):
                """Transpose batch-major h_in (x rows already merged) to the
                feature-major hT used by the GRU matmuls."""
                # overwrite x one-hot + ones columns (baseline data path)
                for bt in range(NBT):
                    eng = nc.vector if bt == 0 else nc.scalar
                    if eng is nc.vector:
                        eng.tensor_copy(
                            hin[:, bt, 501:510],
                            xh[:, (v * NBT + bt) * 9:(v * NBT + bt) * 9 + 9])
                    else:
                        eng.copy(
                            hin[:, bt, 501:510],
                            xh[:, (v * NBT + bt) * 9:(v * NBT + bt) * 9 + 9])
                ptp = ptrp.tile([128, 2, 4, 128], BF, tag="ptr", name="ptp")
                for bt in range(NBT):
                    for kc in range(4):
                        nc.tensor.transpose(
                            ptp[:, bt, kc, :],
                            hin[:, bt, kc * 128:(kc + 1) * 128], ident[:])
                t = hbuf.tile([128, 4, 256], BF, tag="hT", name="hT")
                for bt in range(NBT):
                    # kc-split across DVE and Scalar so neither queue blocks
                    nc.vector.tensor_copy(t[:, 0:2, bt * 128:(bt + 1) * 128],
                                          ptp[:, bt, 0:2, :])
                    nc.scalar.copy(t[:, 2:4, bt * 128:(bt + 1) * 128],
                                   ptp[:, bt, 2:4, :])
                if FP8_WA:
                    t8 = hbuf.tile([128, 4, 256], F8, tag="hT8", name="hT8")
                    nc.vector.tensor_copy(t8[:, 0:2, :], t[:, 0:2, :])
                    nc.vector.tensor_copy(t8[:, 2:4, :], t[:, 2:4, :])
                else:
                    t8 = t
                return t, t8

            hin0 = work.tile([128, 2, 512], BF, tag="hin", name="hin")
            nc.vector.memset(hin0[:], 0.0)
            hT, hT8 = make_hT(0, hin0)

            # pagg: one live PSUM accumulator for vertex vn, filled entirely
            # during step vn-1 in three slices that plug the PE's wait
            # windows (mid-step, post-WG/WM, and the final msg term).
            def agg_emit(p, vn, us, is_final):
                if vn >= MAX_N or not us:
                    return
                for bt in range(NBT):
                    for u in us:
                        nc.tensor.matmul(
                            p[:, bt, :, :], mask_ap(vn, u, bt),
                            msb[:, u, bt, :],
                            start=(u == 0), stop=(is_final and u == us[-1]),
                            skip_group_check=True)

            for v in range(MAX_N):
                # ---- GRU GEMMs in two mt-pair waves ----
                rz = []
                pbcs = []
                for mtp in range(2):
                    pa = gps.tile([128, 2, 2, 256], F32, tag="gemm",
                                  name="pa")
                    # region-major: accumulation groups sharing a PSUM bank
                    # must not interleave (start=True clears the whole
                    # bank's has_written bits)
                    for mt2 in range(2):
                        mt = 2 * mtp + mt2
                        for half in range(2):
                            co = half * 512 + mt * 128
                            if FP8_WA:
                                for i in range(2):
                                    nc.tensor.matmul(
                                        pa[:, mt2, half, :],
                                        wa[:, 2 * i:2 * i + 2, co:co + 128],
                                        hT8[:, 2 * i:2 * i + 2, :],
                                        start=(i == 0), stop=(i == 1),
                                        perf_mode=DR)
                            else:
                                for kc in range(4):
                                    nc.tensor.matmul(
                                        pa[:, mt2, half, :],
                                        wa[:, kc, co:co + 128],
                                        hT[:, kc, :],
                                        start=(kc == 0), stop=(kc == 3))
                    r_t = work.tile([128, 2, 2, 256], BF, tag=f"rz{mtp}",
                                    name="rz")
                    nc.scalar.activation(r_t[:], pa[:], Sig)
                    rz.append(r_t)

                    pbc = gps.tile([128, 2, 2, 256], F32, tag="gemm",
                                   name="pbc")
                    pbcs.append(pbc)
                    for mt2 in range(2):
                        mt = 2 * mtp + mt2
                        nc.tensor.matmul(
                            pbc[:, mt2, 0, :], wb[:, mt * 128:mt * 128 + 128],
                            hT[:, 3, :], start=True, stop=True)
                        for kc in range(4):
                            nc.tensor.matmul(
                                pbc[:, mt2, 1, :],
                                wc[:, kc, mt * 128:mt * 128 + 128],
                                hT[:, kc, :], start=(kc == 0), stop=(kc == 3))

                # mid-step bubble fill: first slice of the aggregation for
                # the next vertex (PE waits on hv here otherwise)
                vn = v + 1
                pagg = None
                s = 0
                if vn < MAX_N:
                    pagg = paggp.tile([128, 2, 2, 256], F32, tag="pagg",
                                      name="pagg")
                    s = max(0, (2 * (vn - 1)) // 3)
                    agg_emit(pagg, vn, list(range(0, s)), False)

                # ---- n-path pointwise + tanh ----
                n_t = work.tile([128, 4, 256], BF, tag="n_t", name="n_t")
                for mtp in range(2):
                    u_t = work.tile([128, 2, 256], BF, tag=f"u{mtp}",
                                    name="u")
                    t_t = work.tile([128, 2, 256], BF, tag=f"t{mtp}",
                                    name="t")
                    nc.vector.tensor_mul(u_t[:], rz[mtp][:, :, 0, :],
                                         pbcs[mtp][:, :, 1, :])
                    nc.vector.tensor_add(t_t[:], u_t[:],
                                         pbcs[mtp][:, :, 0, :])
                    nc.scalar.activation(n_t[:, 2 * mtp:2 * mtp + 2, :],
                                         t_t[:], Tanh)

                # ---- h' = n + z*(h-n) on DVE ----
                hv = work.tile([128, 4, 256], BF, tag="hv", name="hv")
                d_t = work.tile([128, 4, 256], BF, tag="d_t", name="d_t")
                e_t = work.tile([128, 4, 256], BF, tag="e_t", name="e_t")
                for mtp in range(2):
                    ks = slice(2 * mtp, 2 * mtp + 2)
                    nc.vector.tensor_sub(d_t[:, ks, :], hT[:, ks, :],
                                         n_t[:, ks, :])
                    nc.vector.tensor_mul(e_t[:, ks, :], rz[mtp][:, :, 1, :],
                                         d_t[:, ks, :])
                    nc.vector.tensor_add(hv[:, ks, :], e_t[:, ks, :],
                                         n_t[:, ks, :])

                # ---- gate / mapper GEMMs (kc-outer for early start) ----
                pgm = [gps.tile([128, 2, 2, 256], F32, tag="gemm",
                                name="pgm") for _ in range(2)]
                # region-major (see WA comment): complete each accumulation
                # group before starting its bank-sibling
                for mt in range(4):
                    for kc in range(4):
                        nc.tensor.matmul(
                            pgm[mt // 2][:, mt % 2, 0, :],
                            wg[:, kc, mt * 128:mt * 128 + 128],
                            hv[:, kc, :], start=(kc == 0), stop=(kc == 3))
                    for kc in range(4):
                        nc.tensor.matmul(
                            pgm[mt // 2][:, mt % 2, 1, :],
                            wm[:, kc, mt * 128:mt * 128 + 128],
                            hv[:, kc, :], start=(kc == 0), stop=(kc == 3))

                g_t = work.tile([128, 4, 256], BF, tag="g_t", name="g_t")
                gm = work.tile([128, 4, 256], BF, tag="gm", name="gm")
                for mt in range(4):
                    nc.scalar.activation(
                        g_t[:, mt, :], pgm[mt // 2][:, mt % 2, 0, :], Sig,
                        bias=gb[:, mt * 16 + v:mt * 16 + v + 1])
                for mt in range(4):
                    nc.vector.scalar_tensor_tensor(
                        out=gm[:, mt, :], in0=pgm[mt // 2][:, mt % 2, 1, :],
                        scalar=mb[:, mt * 16 + v:mt * 16 + v + 1],
                        in1=g_t[:, mt, :], op0=ADD, op1=MUL)

                # post-WG/WM fill: remaining prefix terms run while the PE
                # waits for g/gm (they only need messages from steps < v)
                if vn < MAX_N:
                    agg_emit(pagg, vn, list(range(s, vn - 1)), False)

                # ---- transpose msg to batch-major ----
                ptg = ptrp.tile([128, 2, 4, 128], BF, tag="ptr", name="ptg")
                for bt in range(NBT):
                    for mt in range(4):
                        nc.tensor.transpose(
                            ptg[:, bt, mt, :],
                            gm[:, mt, bt * 128:(bt + 1) * 128], ident[:])
                for bt in range(NBT):
                    nc.vector.tensor_copy(msb[:, v, bt, :], ptg[:, bt, :, :])

                if vn < MAX_N:
                    # final aggregation term for vertex vn (uses msg v)
                    agg_emit(pagg, vn, [v], True)

                    # h_in(vn): PSUM -> SBUF split DVE/Scalar so the copies
                    # run in parallel off the critical chain
                    hin = work.tile([128, 2, 512], BF, tag="hin", name="hin")
                    for bt in range(NBT):
                        nc.vector.tensor_copy(hin[:, bt, 0:256],
                                              pagg[:, bt, 0, :])
                        nc.scalar.copy(hin[:, bt, 256:512],
                                       pagg[:, bt, 1, :])
                    hT, hT8 = make_hT(vn, hin)

                # ---- final FC ----
                if v == MAX_N - 1:
                    pf = gps.tile([128, 2, 2, 256], F32, tag="gemm",
                                  name="pf")
                    for kc in range(4):
                        nc.tensor.matmul(
                            pf[:112, 0, 0, :], wf[:, kc, :112],
                            hv[:, kc, :], start=(kc == 0), stop=(kc == 3))
                    out_sb = work.tile([128, 256], F32, tag="out_sb",
                                       name="out_sb")
                    nc.scalar.activation(
                        out_sb[:112, :], pf[:112, 0, 0, :], Ident,
                        bias=fcb[:112, :])
                    nc.sync.dma_start(out=d_y[:], in_=out_sb[:112, :])

    nc.compile()
    return nc


def _prep_static(w_ih, w_hh, b_ih, b_hh, gate_w, gate_b, map_w,
                 fc1_w, fc1_b, fc2_w, fc2_b):
    import ml_dtypes
    f32 = np.float32
    bf16 = ml_dtypes.bfloat16
    fp8 = ml_dtypes.float8_e4m3
    bias = (b_ih + b_hh).astype(f32)
    WA = np.zeros((512, 1024), f32)
    WA[0:501, 0:501] = w_hh[0:501].T
    WA[501:509, 0:501] = w_ih[0:501].T
    WA[509, 0:501] = bias[0:501]
    WA[0:501, 512:1013] = w_hh[501:1002].T
    WA[501:509, 512:1013] = w_ih[501:1002].T
    WA[509, 512:1013] = bias[501:1002]
    WC = np.zeros((512, 512), f32)
    WC[0:501, 0:501] = w_hh[1002:1503].T
    WC[509, 0:501] = b_hh[1002:1503]
    WB = np.zeros((128, 512), f32)
    WB[117:125, 0:501] = w_ih[1002:1503].T
    WB[125, 0:501] = b_ih[1002:1503]
    WG = np.zeros((512, 512), f32)
    WG[0:501, 0:501] = gate_w[:, 0:501].T
    WM = np.zeros((512, 512), f32)
    WM[0:501, 0:501] = map_w[:, 0:501].T
    WF = np.zeros((512, 112), f32)
    WF[0:501, 0:56] = fc1_w.T
    WF[0:501, 56:112] = fc2_w.T

    def ktile_flat(W, cols, dt):
        return np.ascontiguousarray(
            W.reshape(4, 128, cols).transpose(1, 0, 2).reshape(128, 4 * cols)
        ).astype(dt)

    wa = ktile_flat(WA, 1024, fp8 if FP8_WA else bf16)
    wcf = ktile_flat(WC, 512, bf16)
    wgf = ktile_flat(WG, 512, bf16)
    wmf = ktile_flat(WM, 512, bf16)
    wff = ktile_flat(WF, 112, bf16)

    gbm = np.zeros((128, 64), f32)
    mbm = np.zeros((128, 64), f32)
    for mt in range(4):
        f0 = mt * 128
        n_real = max(0, min(128, 501 - f0))
        if n_real > 0:
            rows = np.arange(f0, f0 + n_real)
            gbm[:n_real, mt * 16:(mt + 1) * 16] = (
                gate_b[rows, None] + gate_w[rows, HS:HS + 16])
            mbm[:n_real, mt * 16:(mt + 1) * 16] = map_w[rows, HS:HS + 16]
    fcb = np.zeros((128, 1), f32)
    fcb[0:56, 0] = fc1_b
    fcb[56:112, 0] = fc2_b
    ident = np.eye(128, dtype=np.float32).astype(bf16)
    return dict(wa=wa, wb=WB.astype(bf16), wc=wcf, wg=wgf, wm=wmf,
                wf=wff, gb=gbm, mb=mbm, fcb=fcb, ident=ident)


def _prep_core(node_types, adj, core):
    import ml_dtypes
    f32 = np.float32
    bf16 = ml_dtypes.bfloat16
    off = core * BL
    nt = node_types[off:off + BL]          # [256, 16] int32
    ad = adj[off:off + BL].astype(f32)     # [256, 16, 16]
    # batch-major x blocks: 8 one-hot cols + a ones col per (v, bt)
    xh = np.zeros((128, MAX_N * NBT * 9), f32)
    for bt in range(NBT):
        nb = nt[bt * 128:(bt + 1) * 128]   # [128, 16]
        oh = (nb[:, :, None] == np.arange(NVT)[None, None, :]).astype(f32)
        for v in range(MAX_N):
            base = (v * NBT + bt) * 9
            xh[:, base:base + 8] = oh[:, v, :]
            xh[:, base + 8] = 1.0
    # prebuilt diagonal masks for the PE aggregation
    dm = np.zeros((128, N_MASKS * 128), f32)
    rng = np.arange(128)
    for (w, u, bt), k in MASK_IDX.items():
        dm[rng, k * 128 + rng] = ad[bt * 128:(bt + 1) * 128, w, u]
    return dict(xh=xh.astype(bf16), dmasks=dm.astype(bf16))


def kernel(node_types, adj, w_ih, w_hh, b_ih, b_hh, gate_w, gate_b, map_w,
           fc1_w, fc1_b, fc2_w, fc2_b):
    from concourse.bass_utils import run_bass_kernel_spmd

    if "nc" not in _CACHE:
        _CACHE["nc"] = _build_nc()
    nc = _CACHE["nc"]

    node_types = np.asarray(node_types)
    adj = np.asarray(adj, dtype=np.float32)
    static = _prep_static(
        np.asarray(w_ih, np.float32), np.asarray(w_hh, np.float32),
        np.asarray(b_ih, np.float32), np.asarray(b_hh, np.float32),
        np.asarray(gate_w, np.float32), np.asarray(gate_b, np.float32),
        np.asarray(map_w, np.float32),
        np.asarray(fc1_w, np.float32), np.asarray(fc1_b, np.float32),
        np.asarray(fc2_w, np.float32), np.asarray(fc2_b, np.float32))
    in_maps = []
    for c in range(NC_CORES):
        m = dict(static)
        m.update(_prep_core(node_types, adj, c))
        in_maps.append(m)

    res = run_bass_kernel_spmd(nc, in_maps, core_ids=list(range(NC_CORES)))
    ys = [res.results[c]["y"] for c in range(NC_CORES)]   # each [112, 256]
    out = np.concatenate(ys, axis=1).T                     # [2048, 112]
    return np.ascontiguousarray(out.astype(np.float32))


# revision 32
# speedup vs baseline: 1.0956x; 1.0956x over previous
"""DVAE encoder (batched DAG GRU message passing) on 8 trn2 NeuronCores.

Strategy: pure data-parallel over batch (256 graphs/core). Per core, all
state lives in SBUF. Compute is feature-major (features on partitions,
batch on free dim): GRU/gate/mapper are weight-stationary bf16 matmuls
with N=256 and fp32 PSUM accumulation; the one-hot input and all static
biases are folded into augmented contraction rows of the hidden vector.
Predecessor aggregation also runs on the tensor engine: for each edge
term, h_psum[b,:] += diag(adj[:,v,u]) @ msg_u[b,:], a K=128/N=512 bf16
matmul accumulating in fp32 PSUM; diagonal mask tiles are built by DVE
tensor_scalar (4x mode) from an identity. xbar DMA transposes (on
otherwise-idle DMA engines) bridge feature-major results into the
batch-major message buffer and back.
"""

import numpy as np

B, MAX_N, NVT, HS, NZ = 2048, 16, 8, 501, 56
HP = 512          # padded hidden
NC_CORES = 8
BL = B // NC_CORES  # 256 per core
NBT = BL // 128     # 2 batch tiles

_CACHE = {}


def _build_nc():
    import concourse.mybir as mybir
    import concourse.tile as tile
    from concourse import bacc

    F32 = mybir.dt.float32
    BF = mybir.dt.bfloat16

    nc = bacc.Bacc("TRN2", target_bir_lowering=False, debug=False,
                   num_devices=NC_CORES)

    # ---- DRAM parameters (host-prepped) ----
    d_wa = nc.dram_tensor("wa", [128, 4 * 1024], BF, kind="ExternalInput").ap()
    d_wb = nc.dram_tensor("wb", [128, 512], BF, kind="ExternalInput").ap()
    d_wc = nc.dram_tensor("wc", [128, 4 * 512], BF, kind="ExternalInput").ap()
    d_wg = nc.dram_tensor("wg", [128, 4 * 512], BF, kind="ExternalInput").ap()
    d_wm = nc.dram_tensor("wm", [128, 4 * 512], BF, kind="ExternalInput").ap()
    d_wf = nc.dram_tensor("wf", [128, 4 * 112], BF, kind="ExternalInput").ap()
    d_gb = nc.dram_tensor("gb", [128, 64], F32, kind="ExternalInput").ap()
    d_mb = nc.dram_tensor("mb", [128, 64], F32, kind="ExternalInput").ap()
    d_fcb = nc.dram_tensor("fcb", [128, 1], F32, kind="ExternalInput").ap()
    d_xh = nc.dram_tensor("xh", [128, 16 * NBT * 9], BF, kind="ExternalInput").ap()
    d_adj = nc.dram_tensor("adjf", [128, NBT * 16 * 16], F32, kind="ExternalInput").ap()
    d_id = nc.dram_tensor("ident", [128, 128], BF, kind="ExternalInput").ap()
    d_y = nc.dram_tensor("y", [112, BL], F32, kind="ExternalOutput").ap()

    with tile.TileContext(nc) as tc:
        with tc.tile_pool(name="wts", bufs=1) as wts, \
             tc.tile_pool(name="state", bufs=1) as state, \
             tc.tile_pool(name="hbm", bufs=2) as hbmp, \
             tc.tile_pool(name="dpool", bufs=36) as dpool, \
             tc.tile_pool(name="work", bufs=2) as work, \
             tc.tile_pool(name="ps", bufs=4, space="PSUM") as ps, \
             tc.tile_pool(name="pagg", bufs=2, space="PSUM") as paggp:

            # ---- load static data ----
            wa = wts.tile([128, 4 * 1024], BF, tag="wa", name="wa")
            wb = wts.tile([128, 512], BF, tag="wb", name="wb")
            wc = wts.tile([128, 4 * 512], BF, tag="wc", name="wc")
            wg = wts.tile([128, 4 * 512], BF, tag="wg", name="wg")
            wm = wts.tile([128, 4 * 512], BF, tag="wm", name="wm")
            wf = wts.tile([128, 4 * 112], BF, tag="wf", name="wf")
            gb = wts.tile([128, 64], F32, tag="gb", name="gb")
            mb = wts.tile([128, 64], F32, tag="mb", name="mb")
            fcb = wts.tile([128, 1], F32, tag="fcb", name="fcb")
            xh = wts.tile([128, 16 * NBT * 9], BF, tag="xh", name="xh")
            adjf = wts.tile([128, NBT * 16 * 16], F32, tag="adjf", name="adjf")
            ident = wts.tile([128, 128], BF, tag="ident", name="ident")
            for t, d in ((wa, d_wa), (wb, d_wb), (wc, d_wc), (wg, d_wg),
                         (wm, d_wm), (wf, d_wf), (gb, d_gb), (mb, d_mb),
                         (fcb, d_fcb), (xh, d_xh), (adjf, d_adj), (ident, d_id)):
                nc.sync.dma_start(out=t[:], in_=d[:])

            # messages, batch-major: [128, u(16) * bt(2) * 512]
            msb = state.tile([128, 16 * NBT * 512], BF, tag="msb", name="msb")

            def x_overwrite(hbm, v):
                for bt in range(NBT):
                    nc.vector.tensor_copy(
                        hbm[bt][:, 501:510],
                        xh[:, (v * NBT + bt) * 9:(v * NBT + bt) * 9 + 9])

            # initial h (step 0): no predecessors
            hbm = [hbmp.tile([128, 512], BF, tag=f"hbm{bt}", name=f"hbm{bt}")
                   for bt in range(NBT)]
            for bt in range(NBT):
                nc.vector.memset(hbm[bt][:], 0.0)
            x_overwrite(hbm, 0)

            for v in range(MAX_N):
                # ---- transpose h to feature-major (xbar DMA, 3D out) ----
                hT = work.tile([128, 4, 256], BF, tag="hT", name="hT")
                for bt in range(NBT):
                    ptp = ps.tile([128, 4, 128], BF, tag="ps2", name="ptp",
                                  bufs=2)
                    for kc in range(4):
                        nc.tensor.transpose(
                            ptp[:, kc, :],
                            hbm[bt][:, kc * 128:(kc + 1) * 128], ident[:])
                    nc.vector.tensor_copy(
                        hT[:, :, bt * 128:(bt + 1) * 128], ptp[:])

                # ---- GRU matmuls ----
                pa = []
                for mt in range(4):
                    p = ps.tile([128, 2, 256], F32, tag="ps", name="ps")
                    pa.append(p)
                    for half in range(2):
                        for kc in range(4):
                            nc.tensor.matmul(
                                p[:, half, :],
                                wa[:, kc * 1024 + half * 512 + mt * 128:
                                   kc * 1024 + half * 512 + mt * 128 + 128],
                                hT[:, kc, :],
                                start=(kc == 0), stop=(kc == 3))
                pbc = []
                for mt in range(4):
                    p = ps.tile([128, 2, 256], F32, tag="ps", name="ps")
                    pbc.append(p)
                    # Bn = w_ih_n @ x + b_ih_n  (K-tile 3 only)
                    nc.tensor.matmul(
                        p[:, 0, :], wb[:, mt * 128:mt * 128 + 128],
                        hT[:, 3, :], start=True, stop=True)
                    # Cn = w_hh_n @ h + b_hh_n
                    for kc in range(4):
                        nc.tensor.matmul(
                            p[:, 1, :],
                            wc[:, kc * 512 + mt * 128:kc * 512 + mt * 128 + 128],
                            hT[:, kc, :], start=(kc == 0), stop=(kc == 3))

                # ---- GRU pointwise (feature-major) ----
                r = work.tile([128, 4, 256], BF, tag="r", name="r")
                z = work.tile([128, 4, 256], BF, tag="z", name="z")
                for mt in range(4):
                    nc.scalar.activation(r[:, mt, :], pa[mt][:, 0, :],
                                         mybir.ActivationFunctionType.Sigmoid)
                    nc.scalar.activation(z[:, mt, :], pa[mt][:, 1, :],
                                         mybir.ActivationFunctionType.Sigmoid)
                u_t = work.tile([128, 4, 256], BF, tag="u_t", name="u_t")
                t_t = work.tile([128, 4, 256], BF, tag="t_t", name="t_t")
                for mt in range(4):
                    nc.vector.tensor_mul(u_t[:, mt, :], r[:, mt, :],
                                         pbc[mt][:, 1, :])
                    nc.vector.tensor_add(t_t[:, mt, :], u_t[:, mt, :],
                                         pbc[mt][:, 0, :])
                n_t = work.tile([128, 4, 256], BF, tag="n_t", name="n_t")
                nc.scalar.activation(n_t[:], t_t[:],
                                     mybir.ActivationFunctionType.Tanh)
                d_t = work.tile([128, 4, 256], BF, tag="d_t", name="d_t")
                hv = work.tile([128, 4, 256], BF, tag="hv", name="hv")
                nc.vector.tensor_sub(d_t[:], hT[:], n_t[:])
                nc.vector.tensor_mul(d_t[:], z[:], d_t[:])
                nc.vector.tensor_add(hv[:], d_t[:], n_t[:])

                # ---- gate / mapper matmuls ----
                pg = [ps.tile([128, 2, 256], F32, tag="ps", name="ps")
                      for _ in range(2)]
                pm = [ps.tile([128, 2, 256], F32, tag="ps", name="ps")
                      for _ in range(2)]
                for mt in range(4):
                    for kc in range(4):
                        nc.tensor.matmul(
                            pg[mt // 2][:, mt % 2, :],
                            wg[:, kc * 512 + mt * 128:kc * 512 + mt * 128 + 128],
                            hv[:, kc, :], start=(kc == 0), stop=(kc == 3))
                for mt in range(4):
                    for kc in range(4):
                        nc.tensor.matmul(
                            pm[mt // 2][:, mt % 2, :],
                            wm[:, kc * 512 + mt * 128:kc * 512 + mt * 128 + 128],
                            hv[:, kc, :], start=(kc == 0), stop=(kc == 3))
                g_t = work.tile([128, 4, 256], BF, tag="g_t", name="g_t")
                gm = work.tile([128, 4, 256], BF, tag="gm", name="gm")
                for mt in range(4):
                    nc.scalar.activation(
                        g_t[:, mt, :], pg[mt // 2][:, mt % 2, :],
                        mybir.ActivationFunctionType.Sigmoid,
                        bias=gb[:, mt * 16 + v:mt * 16 + v + 1])
                for mt in range(4):
                    nc.vector.scalar_tensor_tensor(
                        out=gm[:, mt, :], in0=pm[mt // 2][:, mt % 2, :],
                        scalar=mb[:, mt * 16 + v:mt * 16 + v + 1],
                        in1=g_t[:, mt, :],
                        op0=mybir.AluOpType.add, op1=mybir.AluOpType.mult)

                # ---- aggregation prefix for next step (PE diag-matmuls
                # over already-available messages; overlaps the gm wait) ----
                vn = v + 1
                if vn < MAX_N:
                    hbm = [hbmp.tile([128, 512], BF, tag=f"hbm{bt}",
                                     name=f"hbm{bt}") for bt in range(NBT)]
                    pags = []
                    for bt in range(NBT):
                        pag = paggp.tile([128, 512], F32, tag="pagg",
                                         name="pagg")
                        pags.append(pag)
                        for u in range(vn - 1):
                            dmask = dpool.tile([128, 128], BF, tag="dmask",
                                               name="dmask")
                            nc.vector.tensor_scalar_mul(
                                dmask[:], ident[:],
                                adjf[:, (bt * 16 + vn) * 16 + u:
                                     (bt * 16 + vn) * 16 + u + 1])
                            nc.tensor.matmul(
                                pag[:],
                                dmask[:],
                                msb[:, (u * NBT + bt) * 512:
                                    (u * NBT + bt) * 512 + 512],
                                start=(u == 0), stop=False)

                # ---- transpose GM into batch-major message slot ----
                for bt in range(NBT):
                    off = (v * NBT + bt) * 512
                    ptg = ps.tile([128, 4, 128], BF, tag="ps2", name="ptg",
                                  bufs=2)
                    for mt in range(4):
                        nc.tensor.transpose(
                            ptg[:, mt, :], gm[:, mt, bt * 128:(bt + 1) * 128],
                            ident[:])
                    nc.vector.tensor_copy(
                        msb[:, off:off + 512], ptg[:])

                # ---- final aggregation term (this step's message) ----
                if vn < MAX_N:
                    for bt in range(NBT):
                        u = vn - 1
                        dmask = dpool.tile([128, 128], BF, tag="dmask",
                                           name="dmask")
                        nc.vector.tensor_scalar_mul(
                            dmask[:], ident[:],
                            adjf[:, (bt * 16 + vn) * 16 + u:
                                 (bt * 16 + vn) * 16 + u + 1])
                        nc.tensor.matmul(
                            pags[bt][:],
                            dmask[:],
                            msb[:, (u * NBT + bt) * 512:
                                (u * NBT + bt) * 512 + 512],
                            start=(u == 0), stop=True)
                        nc.scalar.copy(hbm[bt][:], pags[bt][:])
                    x_overwrite(hbm, vn)

                # ---- final FC (last step) ----
                if v == MAX_N - 1:
                    pf = ps.tile([128, 2, 256], F32, tag="ps", name="ps")
                    for kc in range(4):
                        nc.tensor.matmul(
                            pf[:112, 0, :], wf[:, kc * 112:kc * 112 + 112],
                            hv[:, kc, :], start=(kc == 0), stop=(kc == 3))
                    out_sb = work.tile([128, 256], F32, tag="out_sb",
                                       name="out_sb")
                    nc.scalar.activation(
                        out_sb[:112, :], pf[:112, 0, :],
                        mybir.ActivationFunctionType.Identity,
                        bias=fcb[:112, :])
                    nc.sync.dma_start(out=d_y[:], in_=out_sb[:112, :])

    nc.compile()
    return nc


def _prep_static(w_ih, w_hh, b_ih, b_hh, gate_w, gate_b, map_w,
                 fc1_w, fc1_b, fc2_w, fc2_b):
    import ml_dtypes
    f32 = np.float32
    bf16 = ml_dtypes.bfloat16
    bias = (b_ih + b_hh).astype(f32)
    WA = np.zeros((512, 1024), f32)
    WA[0:501, 0:501] = w_hh[0:501].T
    WA[501:509, 0:501] = w_ih[0:501].T
    WA[509, 0:501] = bias[0:501]
    WA[0:501, 512:1013] = w_hh[501:1002].T
    WA[501:509, 512:1013] = w_ih[501:1002].T
    WA[509, 512:1013] = bias[501:1002]
    WC = np.zeros((512, 512), f32)
    WC[0:501, 0:501] = w_hh[1002:1503].T
    WC[509, 0:501] = b_hh[1002:1503]
    WB = np.zeros((128, 512), f32)
    WB[117:125, 0:501] = w_ih[1002:1503].T
    WB[125, 0:501] = b_ih[1002:1503]
    WG = np.zeros((512, 512), f32)
    WG[0:501, 0:501] = gate_w[:, 0:501].T
    WM = np.zeros((512, 512), f32)
    WM[0:501, 0:501] = map_w[:, 0:501].T
    WF = np.zeros((512, 112), f32)
    WF[0:501, 0:56] = fc1_w.T
    WF[0:501, 56:112] = fc2_w.T

    # [128, 4*cols] K-tile-major flats for SBUF
    def ktile_flat(W, cols):
        return np.ascontiguousarray(
            W.reshape(4, 128, cols).transpose(1, 0, 2).reshape(128, 4 * cols)
        ).astype(bf16)

    wa = ktile_flat(WA, 1024)
    wcf = ktile_flat(WC, 512)
    wgf = ktile_flat(WG, 512)
    wmf = ktile_flat(WM, 512)
    wff = ktile_flat(WF, 112)

    gbm = np.zeros((128, 64), f32)
    mbm = np.zeros((128, 64), f32)
    for mt in range(4):
        f0 = mt * 128
        n_real = max(0, min(128, 501 - f0))
        if n_real > 0:
            rows = np.arange(f0, f0 + n_real)
            gbm[:n_real, mt * 16:(mt + 1) * 16] = (
                gate_b[rows, None] + gate_w[rows, HS:HS + 16])
            mbm[:n_real, mt * 16:(mt + 1) * 16] = map_w[rows, HS:HS + 16]
    fcb = np.zeros((128, 1), f32)
    fcb[0:56, 0] = fc1_b
    fcb[56:112, 0] = fc2_b
    ident = np.eye(128, dtype=np.float32).astype(bf16)
    return dict(wa=wa, wb=WB.astype(bf16), wc=wcf, wg=wgf, wm=wmf,
                wf=wff, gb=gbm, mb=mbm, fcb=fcb, ident=ident)


def _prep_core(node_types, adj, core):
    import ml_dtypes
    f32 = np.float32
    off = core * BL
    nt = node_types[off:off + BL]          # [256, 16] int32
    ad = adj[off:off + BL].astype(f32)     # [256, 16, 16]
    xh = np.zeros((128, 16 * NBT * 9), f32)
    adjf = np.zeros((128, NBT * 16 * 16), f32)
    for bt in range(NBT):
        nb = nt[bt * 128:(bt + 1) * 128]   # [128, 16]
        oh = (nb[:, :, None] == np.arange(NVT)[None, None, :]).astype(f32)
        for v in range(16):
            base = (v * NBT + bt) * 9
            xh[:, base:base + 8] = oh[:, v, :]
            xh[:, base + 8] = 1.0
        ab = ad[bt * 128:(bt + 1) * 128]   # [128, 16, 16]
        adjf[:, bt * 256:(bt + 1) * 256] = ab.reshape(128, 256)
    return dict(xh=xh.astype(ml_dtypes.bfloat16), adjf=adjf)


def kernel(node_types, adj, w_ih, w_hh, b_ih, b_hh, gate_w, gate_b, map_w,
           fc1_w, fc1_b, fc2_w, fc2_b):
    from concourse.bass_utils import run_bass_kernel_spmd

    if "nc" not in _CACHE:
        _CACHE["nc"] = _build_nc()
    nc = _CACHE["nc"]

    node_types = np.asarray(node_types)
    adj = np.asarray(adj, dtype=np.float32)
    static = _prep_static(
        np.asarray(w_ih, np.float32), np.asarray(w_hh, np.float32),
        np.asarray(b_ih, np.float32), np.asarray(b_hh, np.float32),
        np.asarray(gate_w, np.float32), np.asarray(gate_b, np.float32),
        np.asarray(map_w, np.float32),
        np.asarray(fc1_w, np.float32), np.asarray(fc1_b, np.float32),
        np.asarray(fc2_w, np.float32), np.asarray(fc2_b, np.float32))
    in_maps = []
    for c in range(NC_CORES):
        m = dict(static)
        m.update(_prep_core(node_types, adj, c))
        in_maps.append(m)

    res = run_bass_kernel_spmd(nc, in_maps, core_ids=list(range(NC_CORES)))
    ys = [res.results[c]["y"] for c in range(NC_CORES)]   # each [112, 256]
    out = np.concatenate(ys, axis=1).T                     # [2048, 112]
    return np.ascontiguousarray(out.astype(np.float32))



# revision 33
# speedup vs baseline: 1.1034x; 1.0072x over previous
"""DVAE encoder (batched DAG GRU message passing) on 8 trn2 NeuronCores.

Strategy: pure data-parallel over batch (256 graphs/core). Per core, all
state lives in SBUF. Compute is feature-major (features on partitions,
batch on free dim): GRU/gate/mapper are weight-stationary bf16 matmuls
with N=256 and fp32 PSUM accumulation; the one-hot input and all static
biases are folded into augmented contraction rows of the hidden vector.
Predecessor aggregation also runs on the tensor engine: for each edge
term, h_psum[b,:] += diag(adj[:,v,u]) @ msg_u[b,:], a K=128/N=512 bf16
matmul accumulating in fp32 PSUM; diagonal mask tiles are built by DVE
tensor_scalar (4x mode) from an identity. xbar DMA transposes (on
otherwise-idle DMA engines) bridge feature-major results into the
batch-major message buffer and back.
"""

import numpy as np

B, MAX_N, NVT, HS, NZ = 2048, 16, 8, 501, 56
HP = 512          # padded hidden
NC_CORES = 8
BL = B // NC_CORES  # 256 per core
NBT = BL // 128     # 2 batch tiles

_CACHE = {}


def _build_nc():
    import concourse.mybir as mybir
    import concourse.tile as tile
    from concourse import bacc

    F32 = mybir.dt.float32
    BF = mybir.dt.bfloat16

    nc = bacc.Bacc("TRN2", target_bir_lowering=False, debug=False,
                   num_devices=NC_CORES)

    # ---- DRAM parameters (host-prepped) ----
    d_wa = nc.dram_tensor("wa", [128, 4 * 1024], BF, kind="ExternalInput").ap()
    d_wb = nc.dram_tensor("wb", [128, 512], BF, kind="ExternalInput").ap()
    d_wc = nc.dram_tensor("wc", [128, 4 * 512], BF, kind="ExternalInput").ap()
    d_wg = nc.dram_tensor("wg", [128, 4 * 512], BF, kind="ExternalInput").ap()
    d_wm = nc.dram_tensor("wm", [128, 4 * 512], BF, kind="ExternalInput").ap()
    d_wf = nc.dram_tensor("wf", [128, 4 * 112], BF, kind="ExternalInput").ap()
    d_gb = nc.dram_tensor("gb", [128, 64], F32, kind="ExternalInput").ap()
    d_mb = nc.dram_tensor("mb", [128, 64], F32, kind="ExternalInput").ap()
    d_fcb = nc.dram_tensor("fcb", [128, 1], F32, kind="ExternalInput").ap()
    d_xh = nc.dram_tensor("xh", [128, 16 * NBT * 9], BF, kind="ExternalInput").ap()
    d_adj = nc.dram_tensor("adjf", [128, NBT * 16 * 16], F32, kind="ExternalInput").ap()
    d_id = nc.dram_tensor("ident", [128, 128], BF, kind="ExternalInput").ap()
    d_y = nc.dram_tensor("y", [112, BL], F32, kind="ExternalOutput").ap()

    with tile.TileContext(nc) as tc:
        with tc.tile_pool(name="wts", bufs=1) as wts, \
             tc.tile_pool(name="state", bufs=1) as state, \
             tc.tile_pool(name="hbm", bufs=2) as hbmp, \
             tc.tile_pool(name="dpool", bufs=36) as dpool, \
             tc.tile_pool(name="work", bufs=2) as work, \
             tc.tile_pool(name="ps", bufs=4, space="PSUM") as ps, \
             tc.tile_pool(name="pagg", bufs=2, space="PSUM") as paggp:

            # ---- load static data ----
            wa = wts.tile([128, 4 * 1024], BF, tag="wa", name="wa")
            wb = wts.tile([128, 512], BF, tag="wb", name="wb")
            wc = wts.tile([128, 4 * 512], BF, tag="wc", name="wc")
            wg = wts.tile([128, 4 * 512], BF, tag="wg", name="wg")
            wm = wts.tile([128, 4 * 512], BF, tag="wm", name="wm")
            wf = wts.tile([128, 4 * 112], BF, tag="wf", name="wf")
            gb = wts.tile([128, 64], F32, tag="gb", name="gb")
            mb = wts.tile([128, 64], F32, tag="mb", name="mb")
            fcb = wts.tile([128, 1], F32, tag="fcb", name="fcb")
            xh = wts.tile([128, 16 * NBT * 9], BF, tag="xh", name="xh")
            adjf = wts.tile([128, NBT * 16 * 16], F32, tag="adjf", name="adjf")
            ident = wts.tile([128, 128], BF, tag="ident", name="ident")
            # order by first use: ident/xh feed the step-0 transposes and
            # x-overwrite, wa/wb/wc the first GRU matmuls
            for t, d in ((ident, d_id), (xh, d_xh), (wa, d_wa), (wb, d_wb),
                         (wc, d_wc), (adjf, d_adj), (gb, d_gb), (mb, d_mb),
                         (wg, d_wg), (wm, d_wm), (wf, d_wf), (fcb, d_fcb)):
                nc.sync.dma_start(out=t[:], in_=d[:])

            # messages, batch-major: [128, u(16) * bt(2) * 512]
            msb = state.tile([128, 16 * NBT * 512], BF, tag="msb", name="msb")

            def x_overwrite(hbm, v):
                for bt in range(NBT):
                    nc.vector.tensor_copy(
                        hbm[bt][:, 501:510],
                        xh[:, (v * NBT + bt) * 9:(v * NBT + bt) * 9 + 9])

            # initial h (step 0): no predecessors
            hbm = [hbmp.tile([128, 512], BF, tag=f"hbm{bt}", name=f"hbm{bt}")
                   for bt in range(NBT)]
            for bt in range(NBT):
                nc.vector.memset(hbm[bt][:], 0.0)
            x_overwrite(hbm, 0)

            for v in range(MAX_N):
                # ---- transpose h to feature-major (xbar DMA, 3D out) ----
                hT = work.tile([128, 4, 256], BF, tag="hT", name="hT")
                for bt in range(NBT):
                    ptp = ps.tile([128, 4, 128], BF, tag="ps2", name="ptp",
                                  bufs=2)
                    for kc in range(4):
                        nc.tensor.transpose(
                            ptp[:, kc, :],
                            hbm[bt][:, kc * 128:(kc + 1) * 128], ident[:])
                    nc.vector.tensor_copy(
                        hT[:, :, bt * 128:(bt + 1) * 128], ptp[:])

                # ---- GRU matmuls ----
                pa = []
                for mt in range(4):
                    p = ps.tile([128, 2, 256], F32, tag="ps", name="ps")
                    pa.append(p)
                    for half in range(2):
                        for kc in range(4):
                            nc.tensor.matmul(
                                p[:, half, :],
                                wa[:, kc * 1024 + half * 512 + mt * 128:
                                   kc * 1024 + half * 512 + mt * 128 + 128],
                                hT[:, kc, :],
                                start=(kc == 0), stop=(kc == 3))
                pbc = []
                for mt in range(4):
                    p = ps.tile([128, 2, 256], F32, tag="ps", name="ps")
                    pbc.append(p)
                    # Bn = w_ih_n @ x + b_ih_n  (K-tile 3 only)
                    nc.tensor.matmul(
                        p[:, 0, :], wb[:, mt * 128:mt * 128 + 128],
                        hT[:, 3, :], start=True, stop=True)
                    # Cn = w_hh_n @ h + b_hh_n
                    for kc in range(4):
                        nc.tensor.matmul(
                            p[:, 1, :],
                            wc[:, kc * 512 + mt * 128:kc * 512 + mt * 128 + 128],
                            hT[:, kc, :], start=(kc == 0), stop=(kc == 3))

                # ---- GRU pointwise (feature-major) ----
                r = work.tile([128, 4, 256], BF, tag="r", name="r")
                z = work.tile([128, 4, 256], BF, tag="z", name="z")
                for mt in range(4):
                    nc.scalar.activation(r[:, mt, :], pa[mt][:, 0, :],
                                         mybir.ActivationFunctionType.Sigmoid)
                    nc.scalar.activation(z[:, mt, :], pa[mt][:, 1, :],
                                         mybir.ActivationFunctionType.Sigmoid)
                u_t = work.tile([128, 4, 256], BF, tag="u_t", name="u_t")
                t_t = work.tile([128, 4, 256], BF, tag="t_t", name="t_t")
                for mt in range(4):
                    nc.vector.tensor_mul(u_t[:, mt, :], r[:, mt, :],
                                         pbc[mt][:, 1, :])
                    nc.vector.tensor_add(t_t[:, mt, :], u_t[:, mt, :],
                                         pbc[mt][:, 0, :])
                n_t = work.tile([128, 4, 256], BF, tag="n_t", name="n_t")
                nc.scalar.activation(n_t[:], t_t[:],
                                     mybir.ActivationFunctionType.Tanh)
                d_t = work.tile([128, 4, 256], BF, tag="d_t", name="d_t")
                hv = work.tile([128, 4, 256], BF, tag="hv", name="hv")
                nc.vector.tensor_sub(d_t[:], hT[:], n_t[:])
                nc.vector.tensor_mul(d_t[:], z[:], d_t[:])
                nc.vector.tensor_add(hv[:], d_t[:], n_t[:])

                # ---- gate / mapper matmuls ----
                pg = [ps.tile([128, 2, 256], F32, tag="ps", name="ps")
                      for _ in range(2)]
                pm = [ps.tile([128, 2, 256], F32, tag="ps", name="ps")
                      for _ in range(2)]
                for mt in range(4):
                    for kc in range(4):
                        nc.tensor.matmul(
                            pg[mt // 2][:, mt % 2, :],
                            wg[:, kc * 512 + mt * 128:kc * 512 + mt * 128 + 128],
                            hv[:, kc, :], start=(kc == 0), stop=(kc == 3))
                for mt in range(4):
                    for kc in range(4):
                        nc.tensor.matmul(
                            pm[mt // 2][:, mt % 2, :],
                            wm[:, kc * 512 + mt * 128:kc * 512 + mt * 128 + 128],
                            hv[:, kc, :], start=(kc == 0), stop=(kc == 3))
                g_t = work.tile([128, 4, 256], BF, tag="g_t", name="g_t")
                gm = work.tile([128, 4, 256], BF, tag="gm", name="gm")
                for mt in range(4):
                    nc.scalar.activation(
                        g_t[:, mt, :], pg[mt // 2][:, mt % 2, :],
                        mybir.ActivationFunctionType.Sigmoid,
                        bias=gb[:, mt * 16 + v:mt * 16 + v + 1])
                for mt in range(4):
                    nc.vector.scalar_tensor_tensor(
                        out=gm[:, mt, :], in0=pm[mt // 2][:, mt % 2, :],
                        scalar=mb[:, mt * 16 + v:mt * 16 + v + 1],
                        in1=g_t[:, mt, :],
                        op0=mybir.AluOpType.add, op1=mybir.AluOpType.mult)

                # ---- aggregation prefix for next step (PE diag-matmuls
                # over already-available messages; overlaps the gm wait) ----
                vn = v + 1
                if vn < MAX_N:
                    hbm = [hbmp.tile([128, 512], BF, tag=f"hbm{bt}",
                                     name=f"hbm{bt}") for bt in range(NBT)]
                    pags = []
                    for bt in range(NBT):
                        pag = paggp.tile([128, 512], F32, tag="pagg",
                                         name="pagg")
                        pags.append(pag)
                        for u in range(vn - 1):
                            dmask = dpool.tile([128, 128], BF, tag="dmask",
                                               name="dmask")
                            nc.vector.tensor_scalar_mul(
                                dmask[:], ident[:],
                                adjf[:, (bt * 16 + vn) * 16 + u:
                                     (bt * 16 + vn) * 16 + u + 1])
                            nc.tensor.matmul(
                                pag[:],
                                dmask[:],
                                msb[:, (u * NBT + bt) * 512:
                                    (u * NBT + bt) * 512 + 512],
                                start=(u == 0), stop=False)

                # ---- transpose GM into batch-major message slot ----
                for bt in range(NBT):
                    off = (v * NBT + bt) * 512
                    ptg = ps.tile([128, 4, 128], BF, tag="ps2", name="ptg",
                                  bufs=2)
                    for mt in range(4):
                        nc.tensor.transpose(
                            ptg[:, mt, :], gm[:, mt, bt * 128:(bt + 1) * 128],
                            ident[:])
                    nc.vector.tensor_copy(
                        msb[:, off:off + 512], ptg[:])

                # ---- final aggregation term (this step's message) ----
                if vn < MAX_N:
                    for bt in range(NBT):
                        u = vn - 1
                        dmask = dpool.tile([128, 128], BF, tag="dmask",
                                           name="dmask")
                        nc.vector.tensor_scalar_mul(
                            dmask[:], ident[:],
                            adjf[:, (bt * 16 + vn) * 16 + u:
                                 (bt * 16 + vn) * 16 + u + 1])
                        nc.tensor.matmul(
                            pags[bt][:],
                            dmask[:],
                            msb[:, (u * NBT + bt) * 512:
                                (u * NBT + bt) * 512 + 512],
                            start=(u == 0), stop=True)
                        nc.scalar.copy(hbm[bt][:], pags[bt][:])
                    x_overwrite(hbm, vn)

                # ---- final FC (last step) ----
                if v == MAX_N - 1:
                    pf = ps.tile([128, 2, 256], F32, tag="ps", name="ps")
                    for kc in range(4):
                        nc.tensor.matmul(
                            pf[:112, 0, :], wf[:, kc * 112:kc * 112 + 112],
                            hv[:, kc, :], start=(kc == 0), stop=(kc == 3))
                    out_sb = work.tile([128, 256], F32, tag="out_sb",
                                       name="out_sb")
                    nc.scalar.activation(
                        out_sb[:112, :], pf[:112, 0, :],
                        mybir.ActivationFunctionType.Identity,
                        bias=fcb[:112, :])
                    nc.sync.dma_start(out=d_y[:], in_=out_sb[:112, :])

    nc.compile()
    return nc


def _prep_static(w_ih, w_hh, b_ih, b_hh, gate_w, gate_b, map_w,
                 fc1_w, fc1_b, fc2_w, fc2_b):
    import ml_dtypes
    f32 = np.float32
    bf16 = ml_dtypes.bfloat16
    bias = (b_ih + b_hh).astype(f32)
    WA = np.zeros((512, 1024), f32)
    WA[0:501, 0:501] = w_hh[0:501].T
    WA[501:509, 0:501] = w_ih[0:501].T
    WA[509, 0:501] = bias[0:501]
    WA[0:501, 512:1013] = w_hh[501:1002].T
    WA[501:509, 512:1013] = w_ih[501:1002].T
    WA[509, 512:1013] = bias[501:1002]
    WC = np.zeros((512, 512), f32)
    WC[0:501, 0:501] = w_hh[1002:1503].T
    WC[509, 0:501] = b_hh[1002:1503]
    WB = np.zeros((128, 512), f32)
    WB[117:125, 0:501] = w_ih[1002:1503].T
    WB[125, 0:501] = b_ih[1002:1503]
    WG = np.zeros((512, 512), f32)
    WG[0:501, 0:501] = gate_w[:, 0:501].T
    WM = np.zeros((512, 512), f32)
    WM[0:501, 0:501] = map_w[:, 0:501].T
    WF = np.zeros((512, 112), f32)
    WF[0:501, 0:56] = fc1_w.T
    WF[0:501, 56:112] = fc2_w.T

    # [128, 4*cols] K-tile-major flats for SBUF
    def ktile_flat(W, cols):
        return np.ascontiguousarray(
            W.reshape(4, 128, cols).transpose(1, 0, 2).reshape(128, 4 * cols)
        ).astype(bf16)

    wa = ktile_flat(WA, 1024)
    wcf = ktile_flat(WC, 512)
    wgf = ktile_flat(WG, 512)
    wmf = ktile_flat(WM, 512)
    wff = ktile_flat(WF, 112)

    gbm = np.zeros((128, 64), f32)
    mbm = np.zeros((128, 64), f32)
    for mt in range(4):
        f0 = mt * 128
        n_real = max(0, min(128, 501 - f0))
        if n_real > 0:
            rows = np.arange(f0, f0 + n_real)
            gbm[:n_real, mt * 16:(mt + 1) * 16] = (
                gate_b[rows, None] + gate_w[rows, HS:HS + 16])
            mbm[:n_real, mt * 16:(mt + 1) * 16] = map_w[rows, HS:HS + 16]
    fcb = np.zeros((128, 1), f32)
    fcb[0:56, 0] = fc1_b
    fcb[56:112, 0] = fc2_b
    ident = np.eye(128, dtype=np.float32).astype(bf16)
    return dict(wa=wa, wb=WB.astype(bf16), wc=wcf, wg=wgf, wm=wmf,
                wf=wff, gb=gbm, mb=mbm, fcb=fcb, ident=ident)


def _prep_core(node_types, adj, core):
    import ml_dtypes
    f32 = np.float32
    off = core * BL
    nt = node_types[off:off + BL]          # [256, 16] int32
    ad = adj[off:off + BL].astype(f32)     # [256, 16, 16]
    xh = np.zeros((128, 16 * NBT * 9), f32)
    adjf = np.zeros((128, NBT * 16 * 16), f32)
    for bt in range(NBT):
        nb = nt[bt * 128:(bt + 1) * 128]   # [128, 16]
        oh = (nb[:, :, None] == np.arange(NVT)[None, None, :]).astype(f32)
        for v in range(16):
            base = (v * NBT + bt) * 9
            xh[:, base:base + 8] = oh[:, v, :]
            xh[:, base + 8] = 1.0
        ab = ad[bt * 128:(bt + 1) * 128]   # [128, 16, 16]
        adjf[:, bt * 256:(bt + 1) * 256] = ab.reshape(128, 256)
    return dict(xh=xh.astype(ml_dtypes.bfloat16), adjf=adjf)


def kernel(node_types, adj, w_ih, w_hh, b_ih, b_hh, gate_w, gate_b, map_w,
           fc1_w, fc1_b, fc2_w, fc2_b):
    from concourse.bass_utils import run_bass_kernel_spmd

    if "nc" not in _CACHE:
        _CACHE["nc"] = _build_nc()
    nc = _CACHE["nc"]

    node_types = np.asarray(node_types)
    adj = np.asarray(adj, dtype=np.float32)
    static = _prep_static(
        np.asarray(w_ih, np.float32), np.asarray(w_hh, np.float32),
        np.asarray(b_ih, np.float32), np.asarray(b_hh, np.float32),
        np.asarray(gate_w, np.float32), np.asarray(gate_b, np.float32),
        np.asarray(map_w, np.float32),
        np.asarray(fc1_w, np.float32), np.asarray(fc1_b, np.float32),
        np.asarray(fc2_w, np.float32), np.asarray(fc2_b, np.float32))
    in_maps = []
    for c in range(NC_CORES):
        m = dict(static)
        m.update(_prep_core(node_types, adj, c))
        in_maps.append(m)

    res = run_bass_kernel_spmd(nc, in_maps, core_ids=list(range(NC_CORES)))
    ys = [res.results[c]["y"] for c in range(NC_CORES)]   # each [112, 256]
    out = np.concatenate(ys, axis=1).T                     # [2048, 112]
    return np.ascontiguousarray(out.astype(np.float32))



# revision 41
# speedup vs baseline: 1.1434x; 1.0362x over previous
"""DVAE encoder (batched DAG GRU message passing) on 8 trn2 NeuronCores.

Strategy: pure data-parallel over batch (256 graphs/core). Per core, all
state lives in SBUF. Compute is feature-major (features on partitions,
batch on free dim): GRU/gate/mapper are weight-stationary bf16 matmuls
with N=256 and fp32 PSUM accumulation; the one-hot input and all static
biases are folded into augmented contraction rows of the hidden vector.
Predecessor aggregation also runs on the tensor engine: for each edge
term, h_psum[b,:] += diag(adj[:,v,u]) @ msg_u[b,:], a K=128/N=512 bf16
matmul accumulating in fp32 PSUM; diagonal mask tiles are built by DVE
tensor_scalar (4x mode) from an identity. xbar DMA transposes (on
otherwise-idle DMA engines) bridge feature-major results into the
batch-major message buffer and back.
"""

import numpy as np

B, MAX_N, NVT, HS, NZ = 2048, 16, 8, 501, 56
HP = 512          # padded hidden
NC_CORES = 8
BL = B // NC_CORES  # 256 per core
NBT = BL // 128     # 2 batch tiles

_CACHE = {}


# host-side mask index: (w, u, bt) -> column block, w-major so the DMA
# arrives in first-use order
def _mask_index():
    idx = {}
    k = 0
    for w in range(1, MAX_N):
        for u in range(w):
            for bt in range(NBT):
                idx[(w, u, bt)] = k
                k += 1
    return idx, k

MASK_IDX, N_MASKS = _mask_index()


def _build_nc():
    import concourse.mybir as mybir
    import concourse.tile as tile
    from concourse import bacc

    F32 = mybir.dt.float32
    BF = mybir.dt.bfloat16

    nc = bacc.Bacc("TRN2", target_bir_lowering=False, debug=False,
                   num_devices=NC_CORES)

    # ---- DRAM parameters (host-prepped) ----
    d_wa = nc.dram_tensor("wa", [128, 4 * 1024], BF, kind="ExternalInput").ap()
    d_wb = nc.dram_tensor("wb", [128, 512], BF, kind="ExternalInput").ap()
    d_wc = nc.dram_tensor("wc", [128, 4 * 512], BF, kind="ExternalInput").ap()
    d_wg = nc.dram_tensor("wg", [128, 4 * 512], BF, kind="ExternalInput").ap()
    d_wm = nc.dram_tensor("wm", [128, 4 * 512], BF, kind="ExternalInput").ap()
    d_wf = nc.dram_tensor("wf", [128, 4 * 112], BF, kind="ExternalInput").ap()
    d_gb = nc.dram_tensor("gb", [128, 64], F32, kind="ExternalInput").ap()
    d_mb = nc.dram_tensor("mb", [128, 64], F32, kind="ExternalInput").ap()
    d_fcb = nc.dram_tensor("fcb", [128, 1], F32, kind="ExternalInput").ap()
    d_xh = nc.dram_tensor("xh", [128, 16 * NBT * 9], BF, kind="ExternalInput").ap()
    d_adj = nc.dram_tensor("adjf", [128, NBT * 16 * 16], F32, kind="ExternalInput").ap()
    d_dm = nc.dram_tensor("dmasks", [128, N_MASKS * 128], BF,
                          kind="ExternalInput").ap()
    d_id = nc.dram_tensor("ident", [128, 128], BF, kind="ExternalInput").ap()
    d_y = nc.dram_tensor("y", [112, BL], F32, kind="ExternalOutput").ap()

    with tile.TileContext(nc) as tc:
        with tc.tile_pool(name="wts", bufs=1) as wts, \
             tc.tile_pool(name="state", bufs=1) as state, \
             tc.tile_pool(name="hbm", bufs=2) as hbmp, \
             tc.tile_pool(name="work", bufs=2) as work, \
             tc.tile_pool(name="ps", bufs=4, space="PSUM") as ps, \
             tc.tile_pool(name="pagg", bufs=2, space="PSUM") as paggp:

            # ---- load static data ----
            wa = wts.tile([128, 4 * 1024], BF, tag="wa", name="wa")
            wb = wts.tile([128, 512], BF, tag="wb", name="wb")
            wc = wts.tile([128, 4 * 512], BF, tag="wc", name="wc")
            wg = wts.tile([128, 4 * 512], BF, tag="wg", name="wg")
            wm = wts.tile([128, 4 * 512], BF, tag="wm", name="wm")
            wf = wts.tile([128, 4 * 112], BF, tag="wf", name="wf")
            gb = wts.tile([128, 64], F32, tag="gb", name="gb")
            mb = wts.tile([128, 64], F32, tag="mb", name="mb")
            fcb = wts.tile([128, 1], F32, tag="fcb", name="fcb")
            xh = wts.tile([128, 16 * NBT * 9], BF, tag="xh", name="xh")
            adjf = wts.tile([128, NBT * 16 * 16], F32, tag="adjf", name="adjf")
            dmasks = wts.tile([128, N_MASKS * 128], BF, tag="dmasks",
                              name="dmasks")
            ident = wts.tile([128, 128], BF, tag="ident", name="ident")
            # order by first use: ident/xh feed the step-0 transposes and
            # x-overwrite, wa/wb/wc the first GRU matmuls
            for t, d in ((ident, d_id), (xh, d_xh), (wa, d_wa), (wb, d_wb),
                         (wc, d_wc), (adjf, d_adj), (gb, d_gb), (mb, d_mb),
                         (wg, d_wg), (wm, d_wm), (wf, d_wf), (fcb, d_fcb)):
                nc.sync.dma_start(out=t[:], in_=d[:])
            # masks arrive in per-vertex chunks so early steps don't wait
            # on the whole 7.9MB transfer
            for w in range(1, MAX_N):
                k0 = MASK_IDX[(w, 0, 0)]
                k1 = MASK_IDX[(w, w - 1, NBT - 1)] + 1
                nc.sync.dma_start(out=dmasks[:, k0 * 128:k1 * 128],
                                  in_=d_dm[:, k0 * 128:k1 * 128])

            def mask_ap(w, u, bt):
                k = MASK_IDX[(w, u, bt)]
                return dmasks[:, k * 128:(k + 1) * 128]

            # messages, batch-major: [128, u(16) * bt(2) * 512]
            msb = state.tile([128, 16 * NBT * 512], BF, tag="msb", name="msb")

            def x_overwrite(hbm, v):
                for bt in range(NBT):
                    nc.vector.tensor_copy(
                        hbm[bt][:, 501:510],
                        xh[:, (v * NBT + bt) * 9:(v * NBT + bt) * 9 + 9])

            # initial h (step 0): no predecessors
            hbm = [hbmp.tile([128, 512], BF, tag=f"hbm{bt}", name=f"hbm{bt}")
                   for bt in range(NBT)]
            for bt in range(NBT):
                nc.vector.memset(hbm[bt][:], 0.0)
            x_overwrite(hbm, 0)

            for v in range(MAX_N):
                # ---- transpose h to feature-major (xbar DMA, 3D out) ----
                hT = work.tile([128, 4, 256], BF, tag="hT", name="hT")
                for bt in range(NBT):
                    ptp = ps.tile([128, 4, 128], BF, tag="ps2", name="ptp",
                                  bufs=2)
                    for kc in range(4):
                        nc.tensor.transpose(
                            ptp[:, kc, :],
                            hbm[bt][:, kc * 128:(kc + 1) * 128], ident[:])
                    nc.vector.tensor_copy(
                        hT[:, :, bt * 128:(bt + 1) * 128], ptp[:])

                # ---- GRU matmuls ----
                pa = []
                for mt in range(4):
                    p = ps.tile([128, 2, 256], F32, tag="ps", name="ps")
                    pa.append(p)
                    for half in range(2):
                        for kc in range(4):
                            nc.tensor.matmul(
                                p[:, half, :],
                                wa[:, kc * 1024 + half * 512 + mt * 128:
                                   kc * 1024 + half * 512 + mt * 128 + 128],
                                hT[:, kc, :],
                                start=(kc == 0), stop=(kc == 3))
                pbc = []
                for mt in range(4):
                    p = ps.tile([128, 2, 256], F32, tag="ps", name="ps")
                    pbc.append(p)
                    # Bn = w_ih_n @ x + b_ih_n  (K-tile 3 only)
                    nc.tensor.matmul(
                        p[:, 0, :], wb[:, mt * 128:mt * 128 + 128],
                        hT[:, 3, :], start=True, stop=True)
                    # Cn = w_hh_n @ h + b_hh_n
                    for kc in range(4):
                        nc.tensor.matmul(
                            p[:, 1, :],
                            wc[:, kc * 512 + mt * 128:kc * 512 + mt * 128 + 128],
                            hT[:, kc, :], start=(kc == 0), stop=(kc == 3))

                # ---- aggregation part A (host-prebuilt masks): fills the
                # PE while it waits on the r/z/n/h' pointwise chain ----
                vn = v + 1
                pags = None
                s = 0
                if vn < MAX_N:
                    pags = [paggp.tile([128, 512], F32, tag="pagg",
                                       name="pagg") for _ in range(NBT)]
                    s = (vn - 1) // 2
                    for bt in range(NBT):
                        for u in range(s):
                            nc.tensor.matmul(
                                pags[bt][:], mask_ap(vn, u, bt),
                                msb[:, (u * NBT + bt) * 512:
                                    (u * NBT + bt) * 512 + 512],
                                start=(u == 0), stop=False)

                # ---- GRU pointwise (feature-major) ----
                r = work.tile([128, 4, 256], BF, tag="r", name="r")
                z = work.tile([128, 4, 256], BF, tag="z", name="z")
                for mt in range(4):
                    nc.scalar.activation(r[:, mt, :], pa[mt][:, 0, :],
                                         mybir.ActivationFunctionType.Sigmoid)
                    nc.scalar.activation(z[:, mt, :], pa[mt][:, 1, :],
                                         mybir.ActivationFunctionType.Sigmoid)
                u_t = work.tile([128, 4, 256], BF, tag="u_t", name="u_t")
                t_t = work.tile([128, 4, 256], BF, tag="t_t", name="t_t")
                for mt in range(4):
                    nc.vector.tensor_mul(u_t[:, mt, :], r[:, mt, :],
                                         pbc[mt][:, 1, :])
                    nc.vector.tensor_add(t_t[:, mt, :], u_t[:, mt, :],
                                         pbc[mt][:, 0, :])
                n_t = work.tile([128, 4, 256], BF, tag="n_t", name="n_t")
                nc.scalar.activation(n_t[:], t_t[:],
                                     mybir.ActivationFunctionType.Tanh)
                d_t = work.tile([128, 4, 256], BF, tag="d_t", name="d_t")
                hv = work.tile([128, 4, 256], BF, tag="hv", name="hv")
                nc.vector.tensor_sub(d_t[:], hT[:], n_t[:])
                nc.vector.tensor_mul(d_t[:], z[:], d_t[:])
                nc.vector.tensor_add(hv[:], d_t[:], n_t[:])

                # ---- gate / mapper matmuls ----
                pg = [ps.tile([128, 2, 256], F32, tag="ps", name="ps")
                      for _ in range(2)]
                pm = [ps.tile([128, 2, 256], F32, tag="ps", name="ps")
                      for _ in range(2)]
                for mt in range(4):
                    for kc in range(4):
                        nc.tensor.matmul(
                            pg[mt // 2][:, mt % 2, :],
                            wg[:, kc * 512 + mt * 128:kc * 512 + mt * 128 + 128],
                            hv[:, kc, :], start=(kc == 0), stop=(kc == 3))
                for mt in range(4):
                    for kc in range(4):
                        nc.tensor.matmul(
                            pm[mt // 2][:, mt % 2, :],
                            wm[:, kc * 512 + mt * 128:kc * 512 + mt * 128 + 128],
                            hv[:, kc, :], start=(kc == 0), stop=(kc == 3))
                g_t = work.tile([128, 4, 256], BF, tag="g_t", name="g_t")
                gm = work.tile([128, 4, 256], BF, tag="gm", name="gm")
                for mt in range(4):
                    nc.scalar.activation(
                        g_t[:, mt, :], pg[mt // 2][:, mt % 2, :],
                        mybir.ActivationFunctionType.Sigmoid,
                        bias=gb[:, mt * 16 + v:mt * 16 + v + 1])
                for mt in range(4):
                    nc.vector.scalar_tensor_tensor(
                        out=gm[:, mt, :], in0=pm[mt // 2][:, mt % 2, :],
                        scalar=mb[:, mt * 16 + v:mt * 16 + v + 1],
                        in1=g_t[:, mt, :],
                        op0=mybir.AluOpType.add, op1=mybir.AluOpType.mult)

                # ---- aggregation part B: remaining prefix terms overlap
                # the gate/mapper + gm wait ----
                if vn < MAX_N:
                    hbm = [hbmp.tile([128, 512], BF, tag=f"hbm{bt}",
                                     name=f"hbm{bt}") for bt in range(NBT)]
                    for bt in range(NBT):
                        for u in range(s, vn - 1):
                            nc.tensor.matmul(
                                pags[bt][:], mask_ap(vn, u, bt),
                                msb[:, (u * NBT + bt) * 512:
                                    (u * NBT + bt) * 512 + 512],
                                start=(u == 0), stop=False)

                # ---- transpose GM into batch-major message slot ----
                for bt in range(NBT):
                    off = (v * NBT + bt) * 512
                    ptg = ps.tile([128, 4, 128], BF, tag="ps2", name="ptg",
                                  bufs=2)
                    for mt in range(4):
                        nc.tensor.transpose(
                            ptg[:, mt, :], gm[:, mt, bt * 128:(bt + 1) * 128],
                            ident[:])
                    nc.vector.tensor_copy(
                        msb[:, off:off + 512], ptg[:])

                # ---- final aggregation term (this step's message) ----
                if vn < MAX_N:
                    for bt in range(NBT):
                        u = vn - 1
                        nc.tensor.matmul(
                            pags[bt][:], mask_ap(vn, u, bt),
                            msb[:, (u * NBT + bt) * 512:
                                (u * NBT + bt) * 512 + 512],
                            start=(u == 0), stop=True)
                        nc.scalar.copy(hbm[bt][:], pags[bt][:])
                    x_overwrite(hbm, vn)

                # ---- final FC (last step) ----
                if v == MAX_N - 1:
                    pf = ps.tile([128, 2, 256], F32, tag="ps", name="ps")
                    for kc in range(4):
                        nc.tensor.matmul(
                            pf[:112, 0, :], wf[:, kc * 112:kc * 112 + 112],
                            hv[:, kc, :], start=(kc == 0), stop=(kc == 3))
                    out_sb = work.tile([128, 256], F32, tag="out_sb",
                                       name="out_sb")
                    nc.scalar.activation(
                        out_sb[:112, :], pf[:112, 0, :],
                        mybir.ActivationFunctionType.Identity,
                        bias=fcb[:112, :])
                    nc.sync.dma_start(out=d_y[:], in_=out_sb[:112, :])

    nc.compile()
    return nc


def _prep_static(w_ih, w_hh, b_ih, b_hh, gate_w, gate_b, map_w,
                 fc1_w, fc1_b, fc2_w, fc2_b):
    import ml_dtypes
    f32 = np.float32
    bf16 = ml_dtypes.bfloat16
    bias = (b_ih + b_hh).astype(f32)
    WA = np.zeros((512, 1024), f32)
    WA[0:501, 0:501] = w_hh[0:501].T
    WA[501:509, 0:501] = w_ih[0:501].T
    WA[509, 0:501] = bias[0:501]
    WA[0:501, 512:1013] = w_hh[501:1002].T
    WA[501:509, 512:1013] = w_ih[501:1002].T
    WA[509, 512:1013] = bias[501:1002]
    WC = np.zeros((512, 512), f32)
    WC[0:501, 0:501] = w_hh[1002:1503].T
    WC[509, 0:501] = b_hh[1002:1503]
    WB = np.zeros((128, 512), f32)
    WB[117:125, 0:501] = w_ih[1002:1503].T
    WB[125, 0:501] = b_ih[1002:1503]
    WG = np.zeros((512, 512), f32)
    WG[0:501, 0:501] = gate_w[:, 0:501].T
    WM = np.zeros((512, 512), f32)
    WM[0:501, 0:501] = map_w[:, 0:501].T
    WF = np.zeros((512, 112), f32)
    WF[0:501, 0:56] = fc1_w.T
    WF[0:501, 56:112] = fc2_w.T

    # [128, 4*cols] K-tile-major flats for SBUF
    def ktile_flat(W, cols):
        return np.ascontiguousarray(
            W.reshape(4, 128, cols).transpose(1, 0, 2).reshape(128, 4 * cols)
        ).astype(bf16)

    wa = ktile_flat(WA, 1024)
    wcf = ktile_flat(WC, 512)
    wgf = ktile_flat(WG, 512)
    wmf = ktile_flat(WM, 512)
    wff = ktile_flat(WF, 112)

    gbm = np.zeros((128, 64), f32)
    mbm = np.zeros((128, 64), f32)
    for mt in range(4):
        f0 = mt * 128
        n_real = max(0, min(128, 501 - f0))
        if n_real > 0:
            rows = np.arange(f0, f0 + n_real)
            gbm[:n_real, mt * 16:(mt + 1) * 16] = (
                gate_b[rows, None] + gate_w[rows, HS:HS + 16])
            mbm[:n_real, mt * 16:(mt + 1) * 16] = map_w[rows, HS:HS + 16]
    fcb = np.zeros((128, 1), f32)
    fcb[0:56, 0] = fc1_b
    fcb[56:112, 0] = fc2_b
    ident = np.eye(128, dtype=np.float32).astype(bf16)
    return dict(wa=wa, wb=WB.astype(bf16), wc=wcf, wg=wgf, wm=wmf,
                wf=wff, gb=gbm, mb=mbm, fcb=fcb, ident=ident)


def _prep_core(node_types, adj, core):
    import ml_dtypes
    f32 = np.float32
    off = core * BL
    nt = node_types[off:off + BL]          # [256, 16] int32
    ad = adj[off:off + BL].astype(f32)     # [256, 16, 16]
    xh = np.zeros((128, 16 * NBT * 9), f32)
    adjf = np.zeros((128, NBT * 16 * 16), f32)
    for bt in range(NBT):
        nb = nt[bt * 128:(bt + 1) * 128]   # [128, 16]
        oh = (nb[:, :, None] == np.arange(NVT)[None, None, :]).astype(f32)
        for v in range(16):
            base = (v * NBT + bt) * 9
            xh[:, base:base + 8] = oh[:, v, :]
            xh[:, base + 8] = 1.0
        ab = ad[bt * 128:(bt + 1) * 128]   # [128, 16, 16]
        adjf[:, bt * 256:(bt + 1) * 256] = ab.reshape(128, 256)
    # prebuilt diagonal masks for the PE aggregation
    dm = np.zeros((128, N_MASKS * 128), f32)
    rng = np.arange(128)
    for (w, u, bt), k in MASK_IDX.items():
        dm[rng, k * 128 + rng] = ad[bt * 128:(bt + 1) * 128, w, u]
    return dict(xh=xh.astype(ml_dtypes.bfloat16), adjf=adjf,
                dmasks=dm.astype(ml_dtypes.bfloat16))


def kernel(node_types, adj, w_ih, w_hh, b_ih, b_hh, gate_w, gate_b, map_w,
           fc1_w, fc1_b, fc2_w, fc2_b):
    from concourse.bass_utils import run_bass_kernel_spmd

    if "nc" not in _CACHE:
        _CACHE["nc"] = _build_nc()
    nc = _CACHE["nc"]

    node_types = np.asarray(node_types)
    adj = np.asarray(adj, dtype=np.float32)
    static = _prep_static(
        np.asarray(w_ih, np.float32), np.asarray(w_hh, np.float32),
        np.asarray(b_ih, np.float32), np.asarray(b_hh, np.float32),
        np.asarray(gate_w, np.float32), np.asarray(gate_b, np.float32),
        np.asarray(map_w, np.float32),
        np.asarray(fc1_w, np.float32), np.asarray(fc1_b, np.float32),
        np.asarray(fc2_w, np.float32), np.asarray(fc2_b, np.float32))
    in_maps = []
    for c in range(NC_CORES):
        m = dict(static)
        m.update(_prep_core(node_types, adj, c))
        in_maps.append(m)

    res = run_bass_kernel_spmd(nc, in_maps, core_ids=list(range(NC_CORES)))
    ys = [res.results[c]["y"] for c in range(NC_CORES)]   # each [112, 256]
    out = np.concatenate(ys, axis=1).T                     # [2048, 112]
    return np.ascontiguousarray(out.astype(np.float32))



# revision 44
# speedup vs baseline: 1.1484x; 1.0044x over previous
"""DVAE encoder (batched DAG GRU message passing) on 8 trn2 NeuronCores.

Strategy: pure data-parallel over batch (256 graphs/core). Per core, all
state lives in SBUF. Compute is feature-major (features on partitions,
batch on free dim): GRU/gate/mapper are weight-stationary bf16 matmuls
with N=256 and fp32 PSUM accumulation; the one-hot input and all static
biases are folded into augmented contraction rows of the hidden vector.
Predecessor aggregation also runs on the tensor engine: for each edge
term, h_psum[b,:] += diag(adj[:,v,u]) @ msg_u[b,:], a K=128/N=512 bf16
matmul accumulating in fp32 PSUM; diagonal mask tiles are built by DVE
tensor_scalar (4x mode) from an identity. xbar DMA transposes (on
otherwise-idle DMA engines) bridge feature-major results into the
batch-major message buffer and back.
"""

import numpy as np

B, MAX_N, NVT, HS, NZ = 2048, 16, 8, 501, 56
HP = 512          # padded hidden
NC_CORES = 8
BL = B // NC_CORES  # 256 per core
NBT = BL // 128     # 2 batch tiles

_CACHE = {}


# host-side mask index: (w, u, bt) -> column block, w-major so the DMA
# arrives in first-use order
def _mask_index():
    idx = {}
    k = 0
    for w in range(1, MAX_N):
        for u in range(w):
            for bt in range(NBT):
                idx[(w, u, bt)] = k
                k += 1
    return idx, k

MASK_IDX, N_MASKS = _mask_index()


def _build_nc():
    import concourse.mybir as mybir
    import concourse.tile as tile
    from concourse import bacc

    F32 = mybir.dt.float32
    BF = mybir.dt.bfloat16

    nc = bacc.Bacc("TRN2", target_bir_lowering=False, debug=False,
                   num_devices=NC_CORES)

    # ---- DRAM parameters (host-prepped) ----
    d_wa = nc.dram_tensor("wa", [128, 4 * 1024], BF, kind="ExternalInput").ap()
    d_wb = nc.dram_tensor("wb", [128, 512], BF, kind="ExternalInput").ap()
    d_wc = nc.dram_tensor("wc", [128, 4 * 512], BF, kind="ExternalInput").ap()
    d_wg = nc.dram_tensor("wg", [128, 4 * 512], BF, kind="ExternalInput").ap()
    d_wm = nc.dram_tensor("wm", [128, 4 * 512], BF, kind="ExternalInput").ap()
    d_wf = nc.dram_tensor("wf", [128, 4 * 112], BF, kind="ExternalInput").ap()
    d_gb = nc.dram_tensor("gb", [128, 64], F32, kind="ExternalInput").ap()
    d_mb = nc.dram_tensor("mb", [128, 64], F32, kind="ExternalInput").ap()
    d_fcb = nc.dram_tensor("fcb", [128, 1], F32, kind="ExternalInput").ap()
    d_xh = nc.dram_tensor("xh", [128, 16 * NBT * 9], BF, kind="ExternalInput").ap()
    d_adj = nc.dram_tensor("adjf", [128, NBT * 16 * 16], F32, kind="ExternalInput").ap()
    d_dm = nc.dram_tensor("dmasks", [128, N_MASKS * 128], BF,
                          kind="ExternalInput").ap()
    d_id = nc.dram_tensor("ident", [128, 128], BF, kind="ExternalInput").ap()
    d_y = nc.dram_tensor("y", [112, BL], F32, kind="ExternalOutput").ap()

    with tile.TileContext(nc) as tc:
        with tc.tile_pool(name="wts", bufs=1) as wts, \
             tc.tile_pool(name="state", bufs=1) as state, \
             tc.tile_pool(name="hbm", bufs=2) as hbmp, \
             tc.tile_pool(name="work", bufs=2) as work, \
             tc.tile_pool(name="ps", bufs=4, space="PSUM") as ps, \
             tc.tile_pool(name="pagg", bufs=2, space="PSUM") as paggp:

            # ---- load static data ----
            wa = wts.tile([128, 4 * 1024], BF, tag="wa", name="wa")
            wb = wts.tile([128, 512], BF, tag="wb", name="wb")
            wc = wts.tile([128, 4 * 512], BF, tag="wc", name="wc")
            wg = wts.tile([128, 4 * 512], BF, tag="wg", name="wg")
            wm = wts.tile([128, 4 * 512], BF, tag="wm", name="wm")
            wf = wts.tile([128, 4 * 112], BF, tag="wf", name="wf")
            gb = wts.tile([128, 64], F32, tag="gb", name="gb")
            mb = wts.tile([128, 64], F32, tag="mb", name="mb")
            fcb = wts.tile([128, 1], F32, tag="fcb", name="fcb")
            xh = wts.tile([128, 16 * NBT * 9], BF, tag="xh", name="xh")
            adjf = wts.tile([128, NBT * 16 * 16], F32, tag="adjf", name="adjf")
            dmasks = wts.tile([128, N_MASKS * 128], BF, tag="dmasks",
                              name="dmasks")
            ident = wts.tile([128, 128], BF, tag="ident", name="ident")
            # order by first use: ident/xh feed the step-0 transposes and
            # x-overwrite, wa/wb/wc the first GRU matmuls
            for t, d in ((ident, d_id), (xh, d_xh), (wa, d_wa), (wb, d_wb),
                         (wc, d_wc), (adjf, d_adj), (gb, d_gb), (mb, d_mb),
                         (wg, d_wg), (wm, d_wm), (wf, d_wf), (fcb, d_fcb)):
                nc.sync.dma_start(out=t[:], in_=d[:])
            # masks arrive in per-vertex chunks so early steps don't wait
            # on the whole 7.9MB transfer
            for w in range(1, MAX_N):
                k0 = MASK_IDX[(w, 0, 0)]
                k1 = MASK_IDX[(w, w - 1, NBT - 1)] + 1
                nc.sync.dma_start(out=dmasks[:, k0 * 128:k1 * 128],
                                  in_=d_dm[:, k0 * 128:k1 * 128])

            def mask_ap(w, u, bt):
                k = MASK_IDX[(w, u, bt)]
                return dmasks[:, k * 128:(k + 1) * 128]

            # messages, batch-major: [128, u(16) * bt(2) * 512]
            msb = state.tile([128, 16 * NBT * 512], BF, tag="msb", name="msb")

            def x_overwrite(hbm, v):
                for bt in range(NBT):
                    nc.vector.tensor_copy(
                        hbm[bt][:, 501:510],
                        xh[:, (v * NBT + bt) * 9:(v * NBT + bt) * 9 + 9])

            # initial h (step 0): no predecessors
            hbm = [hbmp.tile([128, 512], BF, tag=f"hbm{bt}", name=f"hbm{bt}")
                   for bt in range(NBT)]
            for bt in range(NBT):
                nc.vector.memset(hbm[bt][:], 0.0)
            x_overwrite(hbm, 0)

            for v in range(MAX_N):
                # ---- transpose h to feature-major (xbar DMA, 3D out) ----
                hT = work.tile([128, 4, 256], BF, tag="hT", name="hT")
                for bt in range(NBT):
                    ptp = ps.tile([128, 4, 128], BF, tag="ps2", name="ptp",
                                  bufs=2)
                    for kc in range(4):
                        nc.tensor.transpose(
                            ptp[:, kc, :],
                            hbm[bt][:, kc * 128:(kc + 1) * 128], ident[:])
                    nc.vector.tensor_copy(
                        hT[:, :, bt * 128:(bt + 1) * 128], ptp[:])

                # ---- GRU matmuls ----
                pa = []
                for mt in range(4):
                    p = ps.tile([128, 2, 256], F32, tag="ps", name="ps")
                    pa.append(p)
                    for half in range(2):
                        for kc in range(4):
                            nc.tensor.matmul(
                                p[:, half, :],
                                wa[:, kc * 1024 + half * 512 + mt * 128:
                                   kc * 1024 + half * 512 + mt * 128 + 128],
                                hT[:, kc, :],
                                start=(kc == 0), stop=(kc == 3))
                pbc = []
                for mt in range(4):
                    p = ps.tile([128, 2, 256], F32, tag="ps", name="ps")
                    pbc.append(p)
                    # Bn = w_ih_n @ x + b_ih_n  (K-tile 3 only)
                    nc.tensor.matmul(
                        p[:, 0, :], wb[:, mt * 128:mt * 128 + 128],
                        hT[:, 3, :], start=True, stop=True)
                    # Cn = w_hh_n @ h + b_hh_n
                    for kc in range(4):
                        nc.tensor.matmul(
                            p[:, 1, :],
                            wc[:, kc * 512 + mt * 128:kc * 512 + mt * 128 + 128],
                            hT[:, kc, :], start=(kc == 0), stop=(kc == 3))

                # ---- aggregation part A (host-prebuilt masks): fills the
                # PE while it waits on the r/z/n/h' pointwise chain ----
                vn = v + 1
                pags = None
                s = 0
                if vn < MAX_N:
                    pags = [paggp.tile([128, 512], F32, tag="pagg",
                                       name="pagg") for _ in range(NBT)]
                    s = vn - 1
                    for bt in range(NBT):
                        for u in range(s):
                            nc.tensor.matmul(
                                pags[bt][:], mask_ap(vn, u, bt),
                                msb[:, (u * NBT + bt) * 512:
                                    (u * NBT + bt) * 512 + 512],
                                start=(u == 0), stop=False)

                # ---- GRU pointwise (feature-major) ----
                rz = work.tile([128, 4, 2, 256], BF, tag="rz", name="rz")
                for mt in range(4):
                    nc.scalar.activation(rz[:, mt, :, :], pa[mt][:],
                                         mybir.ActivationFunctionType.Sigmoid)
                u_t = work.tile([128, 4, 256], BF, tag="u_t", name="u_t")
                t_t = work.tile([128, 4, 256], BF, tag="t_t", name="t_t")
                for mt in range(4):
                    nc.vector.tensor_mul(u_t[:, mt, :], rz[:, mt, 0, :],
                                         pbc[mt][:, 1, :])
                    nc.vector.tensor_add(t_t[:, mt, :], u_t[:, mt, :],
                                         pbc[mt][:, 0, :])
                n_t = work.tile([128, 4, 256], BF, tag="n_t", name="n_t")
                nc.scalar.activation(n_t[:], t_t[:],
                                     mybir.ActivationFunctionType.Tanh)
                d_t = work.tile([128, 4, 256], BF, tag="d_t", name="d_t")
                hv = work.tile([128, 4, 256], BF, tag="hv", name="hv")
                nc.vector.tensor_sub(d_t[:], hT[:], n_t[:])
                nc.vector.tensor_mul(d_t[:], rz[:, :, 1, :], d_t[:])
                nc.vector.tensor_add(hv[:], d_t[:], n_t[:])

                # ---- gate / mapper matmuls ----
                pg = [ps.tile([128, 2, 256], F32, tag="ps", name="ps")
                      for _ in range(2)]
                pm = [ps.tile([128, 2, 256], F32, tag="ps", name="ps")
                      for _ in range(2)]
                for mt in range(4):
                    for kc in range(4):
                        nc.tensor.matmul(
                            pg[mt // 2][:, mt % 2, :],
                            wg[:, kc * 512 + mt * 128:kc * 512 + mt * 128 + 128],
                            hv[:, kc, :], start=(kc == 0), stop=(kc == 3))
                for mt in range(4):
                    for kc in range(4):
                        nc.tensor.matmul(
                            pm[mt // 2][:, mt % 2, :],
                            wm[:, kc * 512 + mt * 128:kc * 512 + mt * 128 + 128],
                            hv[:, kc, :], start=(kc == 0), stop=(kc == 3))
                g_t = work.tile([128, 4, 256], BF, tag="g_t", name="g_t")
                gm = work.tile([128, 4, 256], BF, tag="gm", name="gm")
                for mt in range(4):
                    nc.scalar.activation(
                        g_t[:, mt, :], pg[mt // 2][:, mt % 2, :],
                        mybir.ActivationFunctionType.Sigmoid,
                        bias=gb[:, mt * 16 + v:mt * 16 + v + 1])
                for mt in range(4):
                    nc.vector.scalar_tensor_tensor(
                        out=gm[:, mt, :], in0=pm[mt // 2][:, mt % 2, :],
                        scalar=mb[:, mt * 16 + v:mt * 16 + v + 1],
                        in1=g_t[:, mt, :],
                        op0=mybir.AluOpType.add, op1=mybir.AluOpType.mult)

                # ---- aggregation part B: remaining prefix terms overlap
                # the gate/mapper + gm wait ----
                if vn < MAX_N:
                    hbm = [hbmp.tile([128, 512], BF, tag=f"hbm{bt}",
                                     name=f"hbm{bt}") for bt in range(NBT)]
                    for bt in range(NBT):
                        for u in range(s, vn - 1):
                            nc.tensor.matmul(
                                pags[bt][:], mask_ap(vn, u, bt),
                                msb[:, (u * NBT + bt) * 512:
                                    (u * NBT + bt) * 512 + 512],
                                start=(u == 0), stop=False)

                # ---- transpose GM into batch-major message slot ----
                for bt in range(NBT):
                    off = (v * NBT + bt) * 512
                    ptg = ps.tile([128, 4, 128], BF, tag="ps2", name="ptg",
                                  bufs=2)
                    for mt in range(4):
                        nc.tensor.transpose(
                            ptg[:, mt, :], gm[:, mt, bt * 128:(bt + 1) * 128],
                            ident[:])
                    nc.vector.tensor_copy(
                        msb[:, off:off + 512], ptg[:])

                # ---- final aggregation term (this step's message) ----
                if vn < MAX_N:
                    for bt in range(NBT):
                        u = vn - 1
                        nc.tensor.matmul(
                            pags[bt][:], mask_ap(vn, u, bt),
                            msb[:, (u * NBT + bt) * 512:
                                (u * NBT + bt) * 512 + 512],
                            start=(u == 0), stop=True)
                        nc.scalar.copy(hbm[bt][:], pags[bt][:])
                    x_overwrite(hbm, vn)

                # ---- final FC (last step) ----
                if v == MAX_N - 1:
                    pf = ps.tile([128, 2, 256], F32, tag="ps", name="ps")
                    for kc in range(4):
                        nc.tensor.matmul(
                            pf[:112, 0, :], wf[:, kc * 112:kc * 112 + 112],
                            hv[:, kc, :], start=(kc == 0), stop=(kc == 3))
                    out_sb = work.tile([128, 256], F32, tag="out_sb",
                                       name="out_sb")
                    nc.scalar.activation(
                        out_sb[:112, :], pf[:112, 0, :],
                        mybir.ActivationFunctionType.Identity,
                        bias=fcb[:112, :])
                    nc.sync.dma_start(out=d_y[:], in_=out_sb[:112, :])

    nc.compile()
    return nc


def _prep_static(w_ih, w_hh, b_ih, b_hh, gate_w, gate_b, map_w,
                 fc1_w, fc1_b, fc2_w, fc2_b):
    import ml_dtypes
    f32 = np.float32
    bf16 = ml_dtypes.bfloat16
    bias = (b_ih + b_hh).astype(f32)
    WA = np.zeros((512, 1024), f32)
    WA[0:501, 0:501] = w_hh[0:501].T
    WA[501:509, 0:501] = w_ih[0:501].T
    WA[509, 0:501] = bias[0:501]
    WA[0:501, 512:1013] = w_hh[501:1002].T
    WA[501:509, 512:1013] = w_ih[501:1002].T
    WA[509, 512:1013] = bias[501:1002]
    WC = np.zeros((512, 512), f32)
    WC[0:501, 0:501] = w_hh[1002:1503].T
    WC[509, 0:501] = b_hh[1002:1503]
    WB = np.zeros((128, 512), f32)
    WB[117:125, 0:501] = w_ih[1002:1503].T
    WB[125, 0:501] = b_ih[1002:1503]
    WG = np.zeros((512, 512), f32)
    WG[0:501, 0:501] = gate_w[:, 0:501].T
    WM = np.zeros((512, 512), f32)
    WM[0:501, 0:501] = map_w[:, 0:501].T
    WF = np.zeros((512, 112), f32)
    WF[0:501, 0:56] = fc1_w.T
    WF[0:501, 56:112] = fc2_w.T

    # [128, 4*cols] K-tile-major flats for SBUF
    def ktile_flat(W, cols):
        return np.ascontiguousarray(
            W.reshape(4, 128, cols).transpose(1, 0, 2).reshape(128, 4 * cols)
        ).astype(bf16)

    wa = ktile_flat(WA, 1024)
    wcf = ktile_flat(WC, 512)
    wgf = ktile_flat(WG, 512)
    wmf = ktile_flat(WM, 512)
    wff = ktile_flat(WF, 112)

    gbm = np.zeros((128, 64), f32)
    mbm = np.zeros((128, 64), f32)
    for mt in range(4):
        f0 = mt * 128
        n_real = max(0, min(128, 501 - f0))
        if n_real > 0:
            rows = np.arange(f0, f0 + n_real)
            gbm[:n_real, mt * 16:(mt + 1) * 16] = (
                gate_b[rows, None] + gate_w[rows, HS:HS + 16])
            mbm[:n_real, mt * 16:(mt + 1) * 16] = map_w[rows, HS:HS + 16]
    fcb = np.zeros((128, 1), f32)
    fcb[0:56, 0] = fc1_b
    fcb[56:112, 0] = fc2_b
    ident = np.eye(128, dtype=np.float32).astype(bf16)
    return dict(wa=wa, wb=WB.astype(bf16), wc=wcf, wg=wgf, wm=wmf,
                wf=wff, gb=gbm, mb=mbm, fcb=fcb, ident=ident)


def _prep_core(node_types, adj, core):
    import ml_dtypes
    f32 = np.float32
    off = core * BL
    nt = node_types[off:off + BL]          # [256, 16] int32
    ad = adj[off:off + BL].astype(f32)     # [256, 16, 16]
    xh = np.zeros((128, 16 * NBT * 9), f32)
    adjf = np.zeros((128, NBT * 16 * 16), f32)
    for bt in range(NBT):
        nb = nt[bt * 128:(bt + 1) * 128]   # [128, 16]
        oh = (nb[:, :, None] == np.arange(NVT)[None, None, :]).astype(f32)
        for v in range(16):
            base = (v * NBT + bt) * 9
            xh[:, base:base + 8] = oh[:, v, :]
            xh[:, base + 8] = 1.0
        ab = ad[bt * 128:(bt + 1) * 128]   # [128, 16, 16]
        adjf[:, bt * 256:(bt + 1) * 256] = ab.reshape(128, 256)
    # prebuilt diagonal masks for the PE aggregation
    dm = np.zeros((128, N_MASKS * 128), f32)
    rng = np.arange(128)
    for (w, u, bt), k in MASK_IDX.items():
        dm[rng, k * 128 + rng] = ad[bt * 128:(bt + 1) * 128, w, u]
    return dict(xh=xh.astype(ml_dtypes.bfloat16), adjf=adjf,
                dmasks=dm.astype(ml_dtypes.bfloat16))


def kernel(node_types, adj, w_ih, w_hh, b_ih, b_hh, gate_w, gate_b, map_w,
           fc1_w, fc1_b, fc2_w, fc2_b):
    from concourse.bass_utils import run_bass_kernel_spmd

    if "nc" not in _CACHE:
        _CACHE["nc"] = _build_nc()
    nc = _CACHE["nc"]

    node_types = np.asarray(node_types)
    adj = np.asarray(adj, dtype=np.float32)
    static = _prep_static(
        np.asarray(w_ih, np.float32), np.asarray(w_hh, np.float32),
        np.asarray(b_ih, np.float32), np.asarray(b_hh, np.float32),
        np.asarray(gate_w, np.float32), np.asarray(gate_b, np.float32),
        np.asarray(map_w, np.float32),
        np.asarray(fc1_w, np.float32), np.asarray(fc1_b, np.float32),
        np.asarray(fc2_w, np.float32), np.asarray(fc2_b, np.float32))
    in_maps = []
    for c in range(NC_CORES):
        m = dict(static)
        m.update(_prep_core(node_types, adj, c))
        in_maps.append(m)

    res = run_bass_kernel_spmd(nc, in_maps, core_ids=list(range(NC_CORES)))
    ys = [res.results[c]["y"] for c in range(NC_CORES)]   # each [112, 256]
    out = np.concatenate(ys, axis=1).T                     # [2048, 112]
    return np.ascontiguousarray(out.astype(np.float32))



# revision 45
# speedup vs baseline: 1.2595x; 1.0967x over previous
"""DVAE encoder (batched DAG GRU message passing) on 8 trn2 NeuronCores.

Strategy: pure data-parallel over batch (256 graphs/core). Per core, all
state lives in SBUF. Compute is feature-major (features on partitions,
batch on free dim): GRU/gate/mapper are weight-stationary bf16 matmuls
with N=256 and fp32 PSUM accumulation; the one-hot input and all static
biases are folded into augmented contraction rows of the hidden vector.
Predecessor aggregation also runs on the tensor engine: for each edge
term, h_psum[b,:] += diag(adj[:,v,u]) @ msg_u[b,:], a K=128/N=512 bf16
matmul accumulating in fp32 PSUM; diagonal mask tiles are built by DVE
tensor_scalar (4x mode) from an identity. xbar DMA transposes (on
otherwise-idle DMA engines) bridge feature-major results into the
batch-major message buffer and back.
"""

import numpy as np

B, MAX_N, NVT, HS, NZ = 2048, 16, 8, 501, 56
HP = 512          # padded hidden
NC_CORES = 8
BL = B // NC_CORES  # 256 per core
NBT = BL // 128     # 2 batch tiles

_CACHE = {}


# host-side mask index: (w, u, bt) -> column block, w-major so the DMA
# arrives in first-use order
def _mask_index():
    idx = {}
    k = 0
    for w in range(1, MAX_N):
        for u in range(w):
            for bt in range(NBT):
                idx[(w, u, bt)] = k
                k += 1
    return idx, k

MASK_IDX, N_MASKS = _mask_index()


def _build_nc():
    import concourse.mybir as mybir
    import concourse.tile as tile
    from concourse import bacc

    F32 = mybir.dt.float32
    BF = mybir.dt.bfloat16

    nc = bacc.Bacc("TRN2", target_bir_lowering=False, debug=False,
                   num_devices=NC_CORES)

    # ---- DRAM parameters (host-prepped) ----
    d_wa = nc.dram_tensor("wa", [128, 4 * 1024], BF, kind="ExternalInput").ap()
    d_wb = nc.dram_tensor("wb", [128, 512], BF, kind="ExternalInput").ap()
    d_wc = nc.dram_tensor("wc", [128, 4 * 512], BF, kind="ExternalInput").ap()
    d_wg = nc.dram_tensor("wg", [128, 4 * 512], BF, kind="ExternalInput").ap()
    d_wm = nc.dram_tensor("wm", [128, 4 * 512], BF, kind="ExternalInput").ap()
    d_wf = nc.dram_tensor("wf", [128, 4 * 112], BF, kind="ExternalInput").ap()
    d_gb = nc.dram_tensor("gb", [128, 64], F32, kind="ExternalInput").ap()
    d_mb = nc.dram_tensor("mb", [128, 64], F32, kind="ExternalInput").ap()
    d_fcb = nc.dram_tensor("fcb", [128, 1], F32, kind="ExternalInput").ap()
    d_xh = nc.dram_tensor("xh", [128, 16 * NBT * 9], BF, kind="ExternalInput").ap()
    d_adj = nc.dram_tensor("adjf", [128, NBT * 16 * 16], F32, kind="ExternalInput").ap()
    d_dm = nc.dram_tensor("dmasks", [128, N_MASKS * 128], BF,
                          kind="ExternalInput").ap()
    d_id = nc.dram_tensor("ident", [128, 128], BF, kind="ExternalInput").ap()
    d_y = nc.dram_tensor("y", [112, BL], F32, kind="ExternalOutput").ap()

    with tile.TileContext(nc) as tc:
        with tc.tile_pool(name="wts", bufs=1) as wts, \
             tc.tile_pool(name="state", bufs=1) as state, \
             tc.tile_pool(name="hbm", bufs=2) as hbmp, \
             tc.tile_pool(name="work", bufs=2) as work, \
             tc.tile_pool(name="ps", bufs=4, space="PSUM") as ps, \
             tc.tile_pool(name="pagg", bufs=2, space="PSUM") as paggp:

            # ---- load static data ----
            wa = wts.tile([128, 4 * 1024], BF, tag="wa", name="wa")
            wb = wts.tile([128, 512], BF, tag="wb", name="wb")
            wc = wts.tile([128, 4 * 512], BF, tag="wc", name="wc")
            wg = wts.tile([128, 4 * 512], BF, tag="wg", name="wg")
            wm = wts.tile([128, 4 * 512], BF, tag="wm", name="wm")
            wf = wts.tile([128, 4 * 112], BF, tag="wf", name="wf")
            gb = wts.tile([128, 64], F32, tag="gb", name="gb")
            mb = wts.tile([128, 64], F32, tag="mb", name="mb")
            fcb = wts.tile([128, 1], F32, tag="fcb", name="fcb")
            xh = wts.tile([128, 16 * NBT * 9], BF, tag="xh", name="xh")
            adjf = wts.tile([128, NBT * 16 * 16], F32, tag="adjf", name="adjf")
            dmasks = wts.tile([128, N_MASKS * 128], BF, tag="dmasks",
                              name="dmasks")
            ident = wts.tile([128, 128], BF, tag="ident", name="ident")
            # order by first use: ident/xh feed the step-0 transposes and
            # x-overwrite, wa/wb/wc the first GRU matmuls
            for t, d in ((ident, d_id), (xh, d_xh), (wa, d_wa), (wb, d_wb),
                         (wc, d_wc), (adjf, d_adj), (gb, d_gb), (mb, d_mb),
                         (wg, d_wg), (wm, d_wm), (wf, d_wf), (fcb, d_fcb)):
                nc.sync.dma_start(out=t[:], in_=d[:])
            # masks arrive in per-vertex chunks so early steps don't wait
            # on the whole 7.9MB transfer
            for w in range(1, MAX_N):
                k0 = MASK_IDX[(w, 0, 0)]
                k1 = MASK_IDX[(w, w - 1, NBT - 1)] + 1
                nc.sync.dma_start(out=dmasks[:, k0 * 128:k1 * 128],
                                  in_=d_dm[:, k0 * 128:k1 * 128])

            def mask_ap(w, u, bt):
                k = MASK_IDX[(w, u, bt)]
                return dmasks[:, k * 128:(k + 1) * 128]

            # messages, batch-major: [128, u(16) * bt(2) * 512]
            msb = state.tile([128, 16 * NBT * 512], BF, tag="msb", name="msb")

            def x_overwrite(hbm, v):
                for bt in range(NBT):
                    nc.vector.tensor_copy(
                        hbm[bt][:, 501:510],
                        xh[:, (v * NBT + bt) * 9:(v * NBT + bt) * 9 + 9])

            # initial h (step 0): no predecessors
            hbm = [hbmp.tile([128, 512], BF, tag=f"hbm{bt}", name=f"hbm{bt}")
                   for bt in range(NBT)]
            for bt in range(NBT):
                nc.vector.memset(hbm[bt][:], 0.0)
            x_overwrite(hbm, 0)

            for v in range(MAX_N):
                # ---- transpose h to feature-major (xbar DMA, 3D out) ----
                hT = work.tile([128, 4, 256], BF, tag="hT", name="hT")
                for bt in range(NBT):
                    ptp = ps.tile([128, 4, 128], BF, tag="ps2", name="ptp",
                                  bufs=2)
                    for kc in range(4):
                        nc.tensor.transpose(
                            ptp[:, kc, :],
                            hbm[bt][:, kc * 128:(kc + 1) * 128], ident[:])
                    nc.vector.tensor_copy(
                        hT[:, :, bt * 128:(bt + 1) * 128], ptp[:])

                # ---- GRU matmuls ----
                pa = []
                for mt in range(4):
                    p = ps.tile([128, 2, 256], F32, tag="ps", name="ps")
                    pa.append(p)
                    for half in range(2):
                        for kc in range(4):
                            nc.tensor.matmul(
                                p[:, half, :],
                                wa[:, kc * 1024 + half * 512 + mt * 128:
                                   kc * 1024 + half * 512 + mt * 128 + 128],
                                hT[:, kc, :],
                                start=(kc == 0), stop=(kc == 3))
                pbc = []
                for mt in range(4):
                    p = ps.tile([128, 2, 256], F32, tag="ps", name="ps")
                    pbc.append(p)
                    # Bn = w_ih_n @ x + b_ih_n  (K-tile 3 only)
                    nc.tensor.matmul(
                        p[:, 0, :], wb[:, mt * 128:mt * 128 + 128],
                        hT[:, 3, :], start=True, stop=True)
                    # Cn = w_hh_n @ h + b_hh_n
                    for kc in range(4):
                        nc.tensor.matmul(
                            p[:, 1, :],
                            wc[:, kc * 512 + mt * 128:kc * 512 + mt * 128 + 128],
                            hT[:, kc, :], start=(kc == 0), stop=(kc == 3))

                # ---- aggregation part A (host-prebuilt masks): fills the
                # PE while it waits on the r/z/n/h' pointwise chain ----
                vn = v + 1
                pags = None
                s = 0
                if vn < MAX_N:
                    pags = [paggp.tile([128, 512], F32, tag="pagg",
                                       name="pagg") for _ in range(NBT)]
                    s = vn - 1
                    for bt in range(NBT):
                        for u in range(s):
                            nc.tensor.matmul(
                                pags[bt][:], mask_ap(vn, u, bt),
                                msb[:, (u * NBT + bt) * 512:
                                    (u * NBT + bt) * 512 + 512],
                                start=(u == 0), stop=False)

                # ---- GRU pointwise (feature-major) ----
                rz = work.tile([128, 4, 2, 256], BF, tag="rz", name="rz")
                for mt in range(4):
                    nc.scalar.activation(rz[:, mt, :, :], pa[mt][:],
                                         mybir.ActivationFunctionType.Sigmoid)
                u_t = work.tile([128, 4, 256], BF, tag="u_t", name="u_t")
                t_t = work.tile([128, 4, 256], BF, tag="t_t", name="t_t")
                for mt in range(4):
                    nc.vector.tensor_mul(u_t[:, mt, :], rz[:, mt, 0, :],
                                         pbc[mt][:, 1, :])
                    nc.vector.tensor_add(t_t[:, mt, :], u_t[:, mt, :],
                                         pbc[mt][:, 0, :])
                # per-mt so mt0..2 finish while WB/WC still runs; only the
                # last mt's tail delays the gate/mapper matmuls
                n_t = work.tile([128, 4, 256], BF, tag="n_t", name="n_t")
                d_t = work.tile([128, 4, 256], BF, tag="d_t", name="d_t")
                hv = work.tile([128, 4, 256], BF, tag="hv", name="hv")
                for mt in range(4):
                    nc.scalar.activation(n_t[:, mt, :], t_t[:, mt, :],
                                         mybir.ActivationFunctionType.Tanh)
                    nc.vector.tensor_sub(d_t[:, mt, :], hT[:, mt, :],
                                         n_t[:, mt, :])
                    nc.vector.tensor_mul(d_t[:, mt, :], rz[:, mt, 1, :],
                                         d_t[:, mt, :])
                    nc.vector.tensor_add(hv[:, mt, :], d_t[:, mt, :],
                                         n_t[:, mt, :])

                # ---- gate / mapper matmuls ----
                pg = [ps.tile([128, 2, 256], F32, tag="ps", name="ps")
                      for _ in range(2)]
                pm = [ps.tile([128, 2, 256], F32, tag="ps", name="ps")
                      for _ in range(2)]
                for mt in range(4):
                    for kc in range(4):
                        nc.tensor.matmul(
                            pg[mt // 2][:, mt % 2, :],
                            wg[:, kc * 512 + mt * 128:kc * 512 + mt * 128 + 128],
                            hv[:, kc, :], start=(kc == 0), stop=(kc == 3))
                for mt in range(4):
                    for kc in range(4):
                        nc.tensor.matmul(
                            pm[mt // 2][:, mt % 2, :],
                            wm[:, kc * 512 + mt * 128:kc * 512 + mt * 128 + 128],
                            hv[:, kc, :], start=(kc == 0), stop=(kc == 3))
                g_t = work.tile([128, 4, 256], BF, tag="g_t", name="g_t")
                gm = work.tile([128, 4, 256], BF, tag="gm", name="gm")
                for mt in range(4):
                    nc.scalar.activation(
                        g_t[:, mt, :], pg[mt // 2][:, mt % 2, :],
                        mybir.ActivationFunctionType.Sigmoid,
                        bias=gb[:, mt * 16 + v:mt * 16 + v + 1])
                for mt in range(4):
                    nc.vector.scalar_tensor_tensor(
                        out=gm[:, mt, :], in0=pm[mt // 2][:, mt % 2, :],
                        scalar=mb[:, mt * 16 + v:mt * 16 + v + 1],
                        in1=g_t[:, mt, :],
                        op0=mybir.AluOpType.add, op1=mybir.AluOpType.mult)

                # ---- aggregation part B: remaining prefix terms overlap
                # the gate/mapper + gm wait ----
                if vn < MAX_N:
                    hbm = [hbmp.tile([128, 512], BF, tag=f"hbm{bt}",
                                     name=f"hbm{bt}") for bt in range(NBT)]
                    for bt in range(NBT):
                        for u in range(s, vn - 1):
                            nc.tensor.matmul(
                                pags[bt][:], mask_ap(vn, u, bt),
                                msb[:, (u * NBT + bt) * 512:
                                    (u * NBT + bt) * 512 + 512],
                                start=(u == 0), stop=False)

                # ---- transpose GM into batch-major message slot ----
                for bt in range(NBT):
                    off = (v * NBT + bt) * 512
                    ptg = ps.tile([128, 4, 128], BF, tag="ps2", name="ptg",
                                  bufs=2)
                    for mt in range(4):
                        nc.tensor.transpose(
                            ptg[:, mt, :], gm[:, mt, bt * 128:(bt + 1) * 128],
                            ident[:])
                    nc.vector.tensor_copy(
                        msb[:, off:off + 512], ptg[:])

                # ---- final aggregation term (this step's message) ----
                if vn < MAX_N:
                    for bt in range(NBT):
                        u = vn - 1
                        nc.tensor.matmul(
                            pags[bt][:], mask_ap(vn, u, bt),
                            msb[:, (u * NBT + bt) * 512:
                                (u * NBT + bt) * 512 + 512],
                            start=(u == 0), stop=True)
                        nc.scalar.copy(hbm[bt][:], pags[bt][:])
                    x_overwrite(hbm, vn)

                # ---- final FC (last step) ----
                if v == MAX_N - 1:
                    pf = ps.tile([128, 2, 256], F32, tag="ps", name="ps")
                    for kc in range(4):
                        nc.tensor.matmul(
                            pf[:112, 0, :], wf[:, kc * 112:kc * 112 + 112],
                            hv[:, kc, :], start=(kc == 0), stop=(kc == 3))
                    out_sb = work.tile([128, 256], F32, tag="out_sb",
                                       name="out_sb")
                    nc.scalar.activation(
                        out_sb[:112, :], pf[:112, 0, :],
                        mybir.ActivationFunctionType.Identity,
                        bias=fcb[:112, :])
                    nc.sync.dma_start(out=d_y[:], in_=out_sb[:112, :])

    nc.compile()
    return nc


def _prep_static(w_ih, w_hh, b_ih, b_hh, gate_w, gate_b, map_w,
                 fc1_w, fc1_b, fc2_w, fc2_b):
    import ml_dtypes
    f32 = np.float32
    bf16 = ml_dtypes.bfloat16
    bias = (b_ih + b_hh).astype(f32)
    WA = np.zeros((512, 1024), f32)
    WA[0:501, 0:501] = w_hh[0:501].T
    WA[501:509, 0:501] = w_ih[0:501].T
    WA[509, 0:501] = bias[0:501]
    WA[0:501, 512:1013] = w_hh[501:1002].T
    WA[501:509, 512:1013] = w_ih[501:1002].T
    WA[509, 512:1013] = bias[501:1002]
    WC = np.zeros((512, 512), f32)
    WC[0:501, 0:501] = w_hh[1002:1503].T
    WC[509, 0:501] = b_hh[1002:1503]
    WB = np.zeros((128, 512), f32)
    WB[117:125, 0:501] = w_ih[1002:1503].T
    WB[125, 0:501] = b_ih[1002:1503]
    WG = np.zeros((512, 512), f32)
    WG[0:501, 0:501] = gate_w[:, 0:501].T
    WM = np.zeros((512, 512), f32)
    WM[0:501, 0:501] = map_w[:, 0:501].T
    WF = np.zeros((512, 112), f32)
    WF[0:501, 0:56] = fc1_w.T
    WF[0:501, 56:112] = fc2_w.T

    # [128, 4*cols] K-tile-major flats for SBUF
    def ktile_flat(W, cols):
        return np.ascontiguousarray(
            W.reshape(4, 128, cols).transpose(1, 0, 2).reshape(128, 4 * cols)
        ).astype(bf16)

    wa = ktile_flat(WA, 1024)
    wcf = ktile_flat(WC, 512)
    wgf = ktile_flat(WG, 512)
    wmf = ktile_flat(WM, 512)
    wff = ktile_flat(WF, 112)

    gbm = np.zeros((128, 64), f32)
    mbm = np.zeros((128, 64), f32)
    for mt in range(4):
        f0 = mt * 128
        n_real = max(0, min(128, 501 - f0))
        if n_real > 0:
            rows = np.arange(f0, f0 + n_real)
            gbm[:n_real, mt * 16:(mt + 1) * 16] = (
                gate_b[rows, None] + gate_w[rows, HS:HS + 16])
            mbm[:n_real, mt * 16:(mt + 1) * 16] = map_w[rows, HS:HS + 16]
    fcb = np.zeros((128, 1), f32)
    fcb[0:56, 0] = fc1_b
    fcb[56:112, 0] = fc2_b
    ident = np.eye(128, dtype=np.float32).astype(bf16)
    return dict(wa=wa, wb=WB.astype(bf16), wc=wcf, wg=wgf, wm=wmf,
                wf=wff, gb=gbm, mb=mbm, fcb=fcb, ident=ident)


def _prep_core(node_types, adj, core):
    import ml_dtypes
    f32 = np.float32
    off = core * BL
    nt = node_types[off:off + BL]          # [256, 16] int32
    ad = adj[off:off + BL].astype(f32)     # [256, 16, 16]
    xh = np.zeros((128, 16 * NBT * 9), f32)
    adjf = np.zeros((128, NBT * 16 * 16), f32)
    for bt in range(NBT):
        nb = nt[bt * 128:(bt + 1) * 128]   # [128, 16]
        oh = (nb[:, :, None] == np.arange(NVT)[None, None, :]).astype(f32)
        for v in range(16):
            base = (v * NBT + bt) * 9
            xh[:, base:base + 8] = oh[:, v, :]
            xh[:, base + 8] = 1.0
        ab = ad[bt * 128:(bt + 1) * 128]   # [128, 16, 16]
        adjf[:, bt * 256:(bt + 1) * 256] = ab.reshape(128, 256)
    # prebuilt diagonal masks for the PE aggregation
    dm = np.zeros((128, N_MASKS * 128), f32)
    rng = np.arange(128)
    for (w, u, bt), k in MASK_IDX.items():
        dm[rng, k * 128 + rng] = ad[bt * 128:(bt + 1) * 128, w, u]
    return dict(xh=xh.astype(ml_dtypes.bfloat16), adjf=adjf,
                dmasks=dm.astype(ml_dtypes.bfloat16))


def kernel(node_types, adj, w_ih, w_hh, b_ih, b_hh, gate_w, gate_b, map_w,
           fc1_w, fc1_b, fc2_w, fc2_b):
    from concourse.bass_utils import run_bass_kernel_spmd

    if "nc" not in _CACHE:
        _CACHE["nc"] = _build_nc()
    nc = _CACHE["nc"]

    node_types = np.asarray(node_types)
    adj = np.asarray(adj, dtype=np.float32)
    static = _prep_static(
        np.asarray(w_ih, np.float32), np.asarray(w_hh, np.float32),
        np.asarray(b_ih, np.float32), np.asarray(b_hh, np.float32),
        np.asarray(gate_w, np.float32), np.asarray(gate_b, np.float32),
        np.asarray(map_w, np.float32),
        np.asarray(fc1_w, np.float32), np.asarray(fc1_b, np.float32),
        np.asarray(fc2_w, np.float32), np.asarray(fc2_b, np.float32))
    in_maps = []
    for c in range(NC_CORES):
        m = dict(static)
        m.update(_prep_core(node_types, adj, c))
        in_maps.append(m)

    res = run_bass_kernel_spmd(nc, in_maps, core_ids=list(range(NC_CORES)))
    ys = [res.results[c]["y"] for c in range(NC_CORES)]   # each [112, 256]
    out = np.concatenate(ys, axis=1).T                     # [2048, 112]
    return np.ascontiguousarray(out.astype(np.float32))

